# revision 1
# baseline (speedup 1.0000x reference)
"""DGCNN forward kernel for Trainium2 (8 NeuronCores, batch-parallel).

Strategy (per core = one sample of the batch):
  - kNN scores S[n,c] = 2<x_n,x_c> - ||x_c||^2 computed directly on PE via an
    augmented matmul  S = [x;1]^T @ [2x;-xx]  (row and column orientations are
    bit-identical, so thresholds are consistent).
  - top-20 per row via 3 rounds of DVE max8/max_index + match_replace.
  - EdgeConv decomposition: y[o,n,j] = u[o, idx[n,j]] + v[o,n] with
    u = wa@x (neighbor part), v = (wb-wa)@x (center part). max_j then needs a
    gather-max over u only; gathers run on GPSIMD (ap_gather), segmented max on
    DVE pools.
  - BN statistics (training-mode, over the whole batch) via mask matmuls on PE:
    s = u'@M^T, q = u'^2@M^T with M[n,c] = [S[n,c] >= t_n], u' mean-centered to
    avoid fp32 catastrophic cancellation; global sums via a tiny AllReduce
    (syncBN) per layer.
  - Final 1x1 conv + BN + leaky relu, stats again via AllReduce.
"""

import numpy as np

B, C0, N = 8, 3, 2048
K = 20
EPS = 1e-5
LAYERS = [(3, 64), (64, 64), (64, 128), (128, 256)]  # (C_in, O)
NT = N // 128          # 16 row tiles
NCH = N // 512         # 4 matmul free-dim chunks
NEG = -1.0e38

_CACHE = {}


def _build():
    import concourse.bass as bass
    import concourse.mybir as mybir
    from concourse import bacc
    from concourse.tile import TileContext

    dt = mybir.dt
    Alu = mybir.AluOpType
    Act = mybir.ActivationFunctionType

    nc = bacc.Bacc("TRN2", target_bir_lowering=False, debug=False,
                   enable_asserts=False, num_devices=8)

    # ---------------- DRAM I/O ----------------
    x_in = nc.dram_tensor("x0", [C0, N], dt.float32, kind="ExternalInput").ap()
    waT, wbmaT, gv, bv = {}, {}, {}, {}
    for li, (C, O) in enumerate(LAYERS):
        waT[li] = nc.dram_tensor(f"waT{li}", [C, O], dt.float32, kind="ExternalInput").ap()
        wbmaT[li] = nc.dram_tensor(f"wbmaT{li}", [C, O], dt.float32, kind="ExternalInput").ap()
        gv[li] = nc.dram_tensor(f"g{li}", [O, 1], dt.float32, kind="ExternalInput").ap()
        bv[li] = nc.dram_tensor(f"b{li}", [O, 1], dt.float32, kind="ExternalInput").ap()
    w5T_d = nc.dram_tensor("w5T", [512, 1024], dt.float32, kind="ExternalInput").ap()
    g5_d = nc.dram_tensor("g5", [1024, 1], dt.float32, kind="ExternalInput").ap()
    b5_d = nc.dram_tensor("b5", [1024, 1], dt.float32, kind="ExternalInput").ap()
    out_d = nc.dram_tensor("out", [1024, N], dt.float32, kind="ExternalOutput").ap()
    dbg_t = nc.dram_tensor("dbg_t", [128, NT], dt.float32, kind="ExternalOutput").ap()
    dbg_sm = nc.dram_tensor("dbg_sm", [128, 96], dt.float32, kind="ExternalOutput").ap()
    dbg_s = nc.dram_tensor("dbg_s", [128, N], dt.float32, kind="ExternalOutput").ap()
    dbg_h = [nc.dram_tensor(f"dbg_h{i}", [128, N], dt.float32, kind="ExternalOutput").ap()
             for i in range(4)]

    def sb(name, shape, dtype=dt.float32):
        return nc.alloc_sbuf_tensor(name, list(shape), dtype).ap()

    with TileContext(nc) as tc:
        # ---------------- persistent SBUF ----------------
        h = [sb("h0", [128, N]), sb("h1", [128, N]),
             sb("h2", [128, N]), sb("h3", [128, N])]
        ones_row = sb("ones_row", [1, N])
        nc.vector.memset(ones_row, 1.0)
        # identity for PE transpose: ident[p, f] = (f - p == 0)
        ident = sb("ident", [128, 128])
        iota_fp = sb("iota_fp", [128, 128], dt.int32)
        nc.gpsimd.iota(iota_fp, pattern=[[1, 128]], base=0, channel_multiplier=-1)
        nc.vector.tensor_scalar(out=ident, in0=iota_fp, scalar1=0, scalar2=None,
                                op0=Alu.is_equal)

        # x of current layer lives in h-slices; layer0 input loaded separately
        x0_sb = sb("x0_sb", [C0, N])
        nc.sync.dma_start(x0_sb, x_in)

        w5T_sb = sb("w5T_sb", [128, 4 * 1024])   # 4 c-blocks side by side
        for cb in range(4):
            nc.sync.dma_start(w5T_sb[:, cb * 1024:(cb + 1) * 1024],
                              w5T_d[cb * 128:(cb + 1) * 128, :])

        # x2 needs its own base-partition-0 tensor (matmul operands must share base)
        x2_sb = sb("x2_sb", [64, N])
        x_of = {0: x0_sb[:, :], 1: h[0][0:64, :], 2: x2_sb[:, :], 3: h[1][:, :]}
        hdst = {0: h[0][0:64, :], 1: x2_sb[:, :], 2: h[1][:, :],
                3: None}  # layer3 output 256ch -> h2,h3

        t_dram = nc.dram_tensor("t_dram", [NT, 128], dt.float32, kind="Internal").ap()

        for li, (C, O) in enumerate(LAYERS):
            xc = x_of[li]
            OT = (O + 127) // 128            # o-tiles
            ow = [min(128, O - ot * 128) for ot in range(OT)]

            with tc.tile_pool(name=f"pp{li}", bufs=1) as pp:
              with tc.tile_pool(name=f"pt{li}", bufs=3, space="PSUM") as psT:
                # ---- augmented operands ----
                b2x = pp.tile([C, N], dt.float32, tag="b2x")
                nc.scalar.mul(b2x, xc, 2.0)
                xsq = pp.tile([C, N], dt.float32, tag="zt")
                nc.vector.tensor_tensor(out=xsq, in0=xc, in1=xc, op=Alu.mult)
                ones_col = pp.tile([C, 1], dt.float32, tag="ones_col")
                nc.vector.memset(ones_col, 1.0)
                bnxx = pp.tile([1, N], dt.float32, tag="bnxx")
                for nch in range(NCH):
                    xxp = psT.tile([1, 512], dt.float32, tag="pt512")
                    nc.tensor.matmul(xxp, lhsT=ones_col, rhs=xsq[:, nch * 512:(nch + 1) * 512],
                                     start=True, stop=True)
                    nc.scalar.mul(bnxx[:, nch * 512:(nch + 1) * 512], xxp, -1.0)

                # ---- u, v, centering ----
                waT_sb = pp.tile([C, O], dt.float32, tag="waT_sb")
                nc.sync.dma_start(waT_sb, waT[li])
                wbmaT_sb = pp.tile([C, O], dt.float32, tag="wbmaT_sb")
                nc.sync.dma_start(wbmaT_sb, wbmaT[li])
                up = pp.tile([128, OT * N], dt.float32, tag="up")       # u' (centered), o-tiles side by side
                vv = pp.tile([128, OT * N], dt.float32, tag="vv")
                sm = pp.tile([128, 96], dt.float32, tag="sm")
                neg_mu = sm[:, 0:2]
                sum_v = sm[:, 2:4]
                sum_u = sm[:, 4:6]
                sum_q = sm[:, 6:8]
                sum_s = sm[:, 8:10]
                svs = sm[:, 10:12]
                sv2 = sm[:, 12:14]
                vbar = sm[:, 14:16]
                beta = sm[:, 16:18]
                t1a = sm[:, 18:20]
                S1p = sm[:, 20:22]
                tA = sm[:, 22:24]
                tBt = sm[:, 24:26]
                S2p = sm[:, 26:28]
                tC = sm[:, 28:30]
                tD = sm[:, 30:32]
                mean = sm[:, 32:34]
                e2 = sm[:, 34:36]
                varp = sm[:, 36:38]
                rec = sm[:, 38:40]
                rsq = sm[:, 40:42]
                aco = sm[:, 42:44]
                bi = sm[:, 44:46]
                gsb = sm[:, 46:48]
                bsb = sm[:, 48:50]
                ar_in = sm[:, 50:54]
                ar_out = sm[:, 54:58]
                for ot in range(OT):
                    w_ = ow[ot]
                    for nch in range(NCH):
                        upp = psT.tile([128, 512], dt.float32, tag="pt512")
                        nc.tensor.matmul(upp[0:w_, :], lhsT=waT_sb[:, ot * 128:ot * 128 + w_],
                                         rhs=xc[:, nch * 512:(nch + 1) * 512], start=True, stop=True)
                        nc.scalar.activation(up[0:w_, ot * N + nch * 512: ot * N + (nch + 1) * 512],
                                             upp[0:w_, :], Act.Copy)
                        vpp = psT.tile([128, 512], dt.float32, tag="pt512")
                        nc.tensor.matmul(vpp[0:w_, :], lhsT=wbmaT_sb[:, ot * 128:ot * 128 + w_],
                                         rhs=xc[:, nch * 512:(nch + 1) * 512], start=True, stop=True)
                        nc.scalar.activation(vv[0:w_, ot * N + nch * 512: ot * N + (nch + 1) * 512],
                                             vpp[0:w_, :], Act.Copy)
                    nc.vector.tensor_reduce(out=sum_u[0:w_, ot:ot + 1],
                                            in_=up[0:w_, ot * N:(ot + 1) * N],
                                            op=Alu.add, axis=mybir.AxisListType.X)
                    nc.vector.tensor_reduce(out=sum_v[0:w_, ot:ot + 1],
                                            in_=vv[0:w_, ot * N:(ot + 1) * N],
                                            op=Alu.add, axis=mybir.AxisListType.X)
                    nc.scalar.mul(neg_mu[0:w_, ot:ot + 1], sum_u[0:w_, ot:ot + 1], -1.0 / N)
                    # center u in place
                    nc.scalar.activation(up[0:w_, ot * N:(ot + 1) * N],
                                         up[0:w_, ot * N:(ot + 1) * N], Act.Identity,
                                         bias=neg_mu[0:w_, ot:ot + 1], scale=1.0)

                # transpose neg_mu -> row [1, O] for augmented uT matmul
                negmu_row = pp.tile([1, OT * 128], dt.float32, tag="negmu_row")
                for ot in range(OT):
                    w_ = ow[ot]
                    tp = psT.tile([128, 128], dt.float32, tag="pt512")
                    nc.tensor.transpose(tp[0:1, 0:w_], neg_mu[0:w_, ot:ot + 1],
                                        ident[0:w_, 0:w_])
                    nc.scalar.activation(negmu_row[:, ot * 128:ot * 128 + w_], tp[0:1, 0:w_], Act.Copy)

                # ---- phase A: scores + topk per row tile ----
                list16 = pp.tile([16, NT * 128], dt.int16, tag="list16")
                list8 = pp.tile([16, NT * 64], dt.int16, tag="list8")
                t_sb_all = pp.tile([128, NT], dt.float32, tag="t_sb_all")
                with tc.tile_pool(name=f"pa{li}", bufs=2) as pa:
                    for rt in range(NT):
                        Ssb = pa.tile([128, N], dt.float32, tag="Ssb")
                        for nch in range(NCH):
                            Spc = psT.tile([128, 512], dt.float32, tag="pt512")
                            nc.tensor.matmul(Spc,
                                             lhsT=xc[:, rt * 128:(rt + 1) * 128],
                                             rhs=b2x[:, nch * 512:(nch + 1) * 512],
                                             start=True, stop=False)
                            nc.tensor.matmul(Spc,
                                             lhsT=ones_row[:, rt * 128:(rt + 1) * 128],
                                             rhs=bnxx[:, nch * 512:(nch + 1) * 512],
                                             start=False, stop=True)
                            nc.scalar.activation(Ssb[:, nch * 512:(nch + 1) * 512], Spc, Act.Copy)
                        V = pa.tile([128, 24], dt.float32, tag="V")
                        I = pa.tile([128, 24], dt.uint16, tag="I")
                        Sw = pa.tile([128, N], dt.float32, tag="Ssb")   # share slots with Ssb
                        nc.vector.max(out=V[:, 0:8], in_=Ssb)
                        nc.vector.max_index(out=I[:, 0:8], in_max=V[:, 0:8], in_values=Ssb)
                        nc.vector.match_replace(out=Sw, in_to_replace=V[:, 0:8],
                                                in_values=Ssb, imm_value=NEG)
                        nc.vector.max(out=V[:, 8:16], in_=Sw)
                        nc.vector.max_index(out=I[:, 8:16], in_max=V[:, 8:16], in_values=Sw)
                        nc.vector.match_replace(out=Sw, in_to_replace=V[:, 8:16],
                                                in_values=Sw, imm_value=NEG)
                        nc.vector.max(out=V[:, 16:24], in_=Sw)
                        nc.vector.max_index(out=I[:, 16:24], in_max=V[:, 16:24], in_values=Sw)
                        # t_n = 20th value
                        nc.vector.tensor_copy(t_sb_all[:, rt:rt + 1], V[:, 19:20])
                        nc.sync.dma_start(t_dram[rt:rt + 1, :], t_sb_all[:, rt:rt + 1])
                        # index lists
                        If = pa.tile([128, 24], dt.float32, tag="If")
                        nc.vector.tensor_copy(If, I)
                        tpa = psT.tile([128, 128], dt.float32, tag="pt512")
                        nc.tensor.transpose(tpa[0:16, :], If[:, 0:16], ident)
                        nc.vector.tensor_copy(list16[:, rt * 128:(rt + 1) * 128], tpa[0:16, :])
                        tpb = psT.tile([128, 128], dt.float32, tag="pt512")
                        nc.tensor.transpose(tpb[0:8, :], If[:, 16:24], ident)
                        st8 = pa.tile([8, 128], dt.int16, tag="st8")
                        for bb in range(2):
                            nc.vector.tensor_copy(st8[:, bb * 64:(bb + 1) * 64],
                                                  tpb[0:8, bb::2])
                            nc.sync.dma_start(
                                list8[bb * 8:(bb + 1) * 8, rt * 64:(rt + 1) * 64],
                                st8[:, bb * 64:(bb + 1) * 64])

                if li == 0:
                    nc.sync.dma_start(dbg_t, t_sb_all)
                # t as a [1, N] row (via DRAM roundtrip); negated for the mask matmul
                t_row = pp.tile([1, N], dt.float32, tag="t_row")
                nc.sync.dma_start(t_row, t_dram.rearrange("a b -> (a b)")[None, :])
                # widen threshold by ~4 ulp (relative) so the boundary element
                # survives the S^T-side recompute's rounding asymmetry
                eta = float(2.0 ** -20)
                tabs = pp.tile([1, N], dt.float32, tag="zt")
                nc.scalar.mul(tabs, t_row, -(1.0 - eta))
                nc.scalar.mul(t_row, t_row, -(1.0 + eta))
                nc.vector.tensor_tensor(out=t_row, in0=t_row, in1=tabs, op=Alu.max)
                neg_t_row = t_row

                # replicate gather lists into all channel groups
                CH = 128 if O >= 128 else 64
                lg1 = pp.tile([CH, NT * 128], dt.int16, tag="lg1")
                lg2 = pp.tile([CH, NT * 64], dt.int16, tag="lg2")
                for gch in range(CH // 16):
                    nc.sync.dma_start(lg1[gch * 16:(gch + 1) * 16, :], list16)
                    nc.sync.dma_start(lg2[gch * 16:(gch + 1) * 16, :], list8)

                # ---- gather-max over u' ----
                mm = pp.tile([128, OT * N], dt.float32, tag="mm")      # m' per o-tile
                with tc.tile_pool(name=f"pg{li}", bufs=2) as pg:
                    for ot in range(OT):
                        w_ = ow[ot]
                        wch = ((w_ + 15) // 16) * 16
                        usrc = up[0:wch, ot * N:(ot + 1) * N]
                        CKR = 64                        # rows per gather chunk
                        for ck in range(N // CKR):
                            g1 = pg.tile([CH, CKR * 16], dt.float32, tag="g1")
                            nc.gpsimd.ap_gather(
                                g1[0:wch, :], usrc,
                                lg1[0:wch, ck * CKR: (ck + 1) * CKR],
                                channels=wch, num_elems=N, d=1, num_idxs=CKR * 16)
                            nc.vector.tensor_reduce(
                                out=mm[0:w_, ot * N + ck * CKR: ot * N + (ck + 1) * CKR],
                                in_=g1[0:w_, :].rearrange("p (n k) -> p n k", k=16),
                                op=Alu.max, axis=mybir.AxisListType.X)
                            g2 = pg.tile([CH, CKR * 8], dt.float32, tag="g2")
                            nc.gpsimd.ap_gather(
                                g2[0:wch, :], usrc,
                                lg2[0:wch, ck * (CKR // 2): (ck + 1) * (CKR // 2)],
                                channels=wch, num_elems=N, d=1, num_idxs=CKR * 8)
                            m2 = pg.tile([128, CKR], dt.float32, tag="g2")
                            nc.vector.tensor_reduce(
                                out=m2[0:w_, :],
                                in_=g2[0:w_, :].rearrange("p (n k) -> p n k", k=8)[:, :, 0:4],
                                op=Alu.max, axis=mybir.AxisListType.X)
                            nc.vector.tensor_tensor(
                                out=mm[0:w_, ot * N + ck * CKR: ot * N + (ck + 1) * CKR],
                                in0=mm[0:w_, ot * N + ck * CKR: ot * N + (ck + 1) * CKR],
                                in1=m2[0:w_, :], op=Alu.max)

                # ---- mask matmuls: s = u'@M^T, q = u'^2@M^T ----
                s_sb = pp.tile([128, OT * N], dt.float32, tag="s_sb")
                with tc.tile_pool(name=f"pm{li}", bufs=2) as pm, \
                     tc.tile_pool(name=f"pq{li}", bufs=1, space="PSUM") as psQ:
                    for nch in range(NCH):
                        sacc = []
                        qacc = []
                        for ot in range(OT):
                            sa = psQ.tile([128, 512], dt.float32, tag=f"sacc{ot}")
                            sacc.append(sa)
                            qa = psQ.tile([128, 512], dt.float32, tag=f"qacc{ot}")
                            qacc.append(qa)
                        for cb in range(NT):
                            # stp = S^T - t  (3-pass accumulation; bit-consistent with row S)
                            stp = psT.tile([128, 512], dt.float32, tag="pt512")
                            nc.tensor.matmul(stp, lhsT=b2x[:, cb * 128:(cb + 1) * 128],
                                             rhs=xc[:, nch * 512:(nch + 1) * 512],
                                             start=True, stop=False)
                            nc.tensor.matmul(stp, lhsT=bnxx[:, cb * 128:(cb + 1) * 128],
                                             rhs=ones_row[:, nch * 512:(nch + 1) * 512],
                                             start=False, stop=False)
                            nc.tensor.matmul(stp, lhsT=ones_row[:, cb * 128:(cb + 1) * 128],
                                             rhs=neg_t_row[:, nch * 512:(nch + 1) * 512],
                                             start=False, stop=True)
                            mt = pm.tile([128, 512], dt.float32, tag="mt")
                            nc.vector.tensor_scalar(out=mt, in0=stp, scalar1=0.0,
                                                    scalar2=None, op0=Alu.is_ge)
                            utp = psT.tile([128, 256], dt.float32, tag="pt512")
                            for ot in range(OT):
                                w_ = ow[ot]
                                nc.tensor.matmul(utp[:, ot * 128:ot * 128 + w_],
                                                 lhsT=xc[:, cb * 128:(cb + 1) * 128],
                                                 rhs=waT_sb[:, ot * 128:ot * 128 + w_],
                                                 start=True, stop=False)
                                nc.tensor.matmul(utp[:, ot * 128:ot * 128 + w_],
                                                 lhsT=ones_row[:, cb * 128:(cb + 1) * 128],
                                                 rhs=negmu_row[:, ot * 128:ot * 128 + w_],
                                                 start=False, stop=True)
                            uts = pm.tile([128, 256], dt.float32, tag="uts")
                            nc.vector.tensor_copy(uts[:, 0:OT * 128], utp[:, 0:OT * 128])
                            ut2 = pm.tile([128, 256], dt.float32, tag="ut2")
                            nc.vector.tensor_tensor(out=ut2[:, 0:OT * 128], in0=uts[:, 0:OT * 128],
                                                    in1=uts[:, 0:OT * 128], op=Alu.mult)
                            for ot in range(OT):
                                w_ = ow[ot]
                                nc.tensor.matmul(sacc[ot][0:w_, :],
                                                 lhsT=uts[:, ot * 128:ot * 128 + w_], rhs=mt,
                                                 start=(cb == 0), stop=(cb == NT - 1))
                                nc.tensor.matmul(qacc[ot][0:w_, :],
                                                 lhsT=ut2[:, ot * 128:ot * 128 + w_], rhs=mt,
                                                 start=(cb == 0), stop=(cb == NT - 1))
                        for ot in range(OT):
                            w_ = ow[ot]
                            nc.scalar.activation(
                                s_sb[0:w_, ot * N + nch * 512: ot * N + (nch + 1) * 512],
                                sacc[ot][0:w_, :], Act.Copy)
                            qpart = pm.tile([128, 512], dt.float32, tag="qpart")
                            nc.scalar.activation(qpart[0:w_, :], qacc[ot][0:w_, :], Act.Copy,
                                                 accum_out=sum_q[0:w_, ot + 0:ot + 1]
                                                 if nch == 0 else None)
                            if nch > 0:
                                tmp1 = pm.tile([128, 1], dt.float32, tag="tmp1")
                                nc.vector.tensor_reduce(out=tmp1[0:w_, :], in_=qpart[0:w_, :],
                                                        op=Alu.add, axis=mybir.AxisListType.X)
                                nc.vector.tensor_tensor(out=sum_q[0:w_, ot:ot + 1],
                                                        in0=sum_q[0:w_, ot:ot + 1],
                                                        in1=tmp1[0:w_, :], op=Alu.add)

                # ---- per-core stat terms + AllReduce ----
                for ot in range(OT):
                    w_ = ow[ot]
                    ssl = s_sb[0:w_, ot * N:(ot + 1) * N]
                    vsl = vv[0:w_, ot * N:(ot + 1) * N]
                    nc.vector.tensor_reduce(out=sum_s[0:w_, ot:ot + 1], in_=ssl,
                                            op=Alu.add, axis=mybir.AxisListType.X)
                    junk = pp.tile([128, N], dt.float32, tag="zt")
                    nc.vector.scalar_tensor_tensor(out=junk[0:w_, :], in0=ssl, scalar=1.0,
                                                   in1=vsl, op0=Alu.mult, op1=Alu.mult,
                                                   accum_out=svs[0:w_, ot:ot + 1])
                    nc.vector.scalar_tensor_tensor(out=junk[0:w_, :], in0=vsl, scalar=1.0,
                                                   in1=vsl, op0=Alu.mult, op1=Alu.mult,
                                                   accum_out=sv2[0:w_, ot:ot + 1])
                    # small [w_,1] algebra on DVE/ACT:
                    nc.scalar.mul(vbar[0:w_, ot:ot + 1], sum_v[0:w_, ot:ot + 1], 1.0 / N)
                    nc.vector.scalar_tensor_tensor(out=beta[0:w_, ot:ot + 1],
                                                   in0=neg_mu[0:w_, ot:ot + 1], scalar=-1.0,
                                                   in1=vbar[0:w_, ot:ot + 1],
                                                   op0=Alu.mult, op1=Alu.add)
                    # S1' = sum_s + K*(sum_v - N*vbar);  sum_v - N*vbar == 0 exactly? keep it:
                    nc.vector.scalar_tensor_tensor(out=t1a[0:w_, ot:ot + 1],
                                                   in0=vbar[0:w_, ot:ot + 1], scalar=-float(N),
                                                   in1=sum_v[0:w_, ot:ot + 1],
                                                   op0=Alu.mult, op1=Alu.add)  # sum_v - N*vbar
                    nc.vector.scalar_tensor_tensor(out=S1p[0:w_, ot:ot + 1],
                                                   in0=t1a[0:w_, ot:ot + 1], scalar=float(K),
                                                   in1=sum_s[0:w_, ot:ot + 1],
                                                   op0=Alu.mult, op1=Alu.add)
                    # S2' = sum_q + 2*(svs - vbar*sum_s) + K*(sv2 - N*vbar^2)
                    nc.vector.tensor_tensor(out=tA[0:w_, ot:ot + 1], in0=vbar[0:w_, ot:ot + 1],
                                            in1=sum_s[0:w_, ot:ot + 1], op=Alu.mult)
                    nc.vector.scalar_tensor_tensor(out=tA[0:w_, ot:ot + 1],
                                                   in0=tA[0:w_, ot:ot + 1], scalar=-1.0,
                                                   in1=svs[0:w_, ot:ot + 1],
                                                   op0=Alu.mult, op1=Alu.add)  # svs - vbar*sum_s
                    nc.vector.tensor_tensor(out=tBt[0:w_, ot:ot + 1], in0=vbar[0:w_, ot:ot + 1],
                                            in1=vbar[0:w_, ot:ot + 1], op=Alu.mult)
                    nc.vector.scalar_tensor_tensor(out=tBt[0:w_, ot:ot + 1],
                                                   in0=tBt[0:w_, ot:ot + 1], scalar=-float(N),
                                                   in1=sv2[0:w_, ot:ot + 1],
                                                   op0=Alu.mult, op1=Alu.add)  # sv2 - N*vbar^2
                    nc.vector.scalar_tensor_tensor(out=S2p[0:w_, ot:ot + 1],
                                                   in0=tA[0:w_, ot:ot + 1], scalar=2.0,
                                                   in1=sum_q[0:w_, ot:ot + 1],
                                                   op0=Alu.mult, op1=Alu.add)
                    nc.vector.scalar_tensor_tensor(out=S2p[0:w_, ot:ot + 1],
                                                   in0=tBt[0:w_, ot:ot + 1], scalar=float(K),
                                                   in1=S2p[0:w_, ot:ot + 1],
                                                   op0=Alu.mult, op1=Alu.add)
                    # t1 = S1' + cnt*beta ; t2 = S2' + 2*beta*S1' + cnt*beta^2
                    cntl = float(N * K)
                    nc.vector.scalar_tensor_tensor(out=ar_in[0:w_, 2 * ot:2 * ot + 1],
                                                   in0=beta[0:w_, ot:ot + 1], scalar=cntl,
                                                   in1=S1p[0:w_, ot:ot + 1],
                                                   op0=Alu.mult, op1=Alu.add)
                    nc.vector.tensor_tensor(out=tC[0:w_, ot:ot + 1], in0=beta[0:w_, ot:ot + 1],
                                            in1=S1p[0:w_, ot:ot + 1], op=Alu.mult)
                    nc.vector.scalar_tensor_tensor(out=tC[0:w_, ot:ot + 1],
                                                   in0=tC[0:w_, ot:ot + 1], scalar=2.0,
                                                   in1=S2p[0:w_, ot:ot + 1],
                                                   op0=Alu.mult, op1=Alu.add)
                    nc.vector.tensor_tensor(out=tD[0:w_, ot:ot + 1], in0=beta[0:w_, ot:ot + 1],
                                            in1=beta[0:w_, ot:ot + 1], op=Alu.mult)
                    nc.vector.scalar_tensor_tensor(out=ar_in[0:w_, 2 * ot + 1:2 * ot + 2],
                                                   in0=tD[0:w_, ot:ot + 1], scalar=cntl,
                                                   in1=tC[0:w_, ot:ot + 1],
                                                   op0=Alu.mult, op1=Alu.add)

                if li == 0:
                    nc.sync.dma_start(dbg_sm, sm)
                    nc.sync.dma_start(dbg_s, s_sb[:, 0:N])
                with tc.tile_pool(name=f"dr{li}", bufs=1, space="DRAM") as dram:
                    ari = dram.tile([128, 2 * OT], dt.float32)
                    aro = dram.tile([128, 2 * OT], dt.float32)
                    nc.sync.dma_start(ari[:], ar_in[:, 0:2 * OT])
                    nc.gpsimd.collective_compute(
                        "AllReduce", Alu.add, replica_groups=[list(range(8))],
                        ins=[ari.opt()], outs=[aro.opt()])
                    nc.sync.dma_start(ar_out[:, 0:2 * OT], aro[:])

                # post-AR: mean/var/scale/bias + activation
                nc.sync.dma_start(gsb[0:ow[0], 0:1], gv[li][0:ow[0], :])
                nc.sync.dma_start(bsb[0:ow[0], 0:1], bv[li][0:ow[0], :])
                if OT > 1:
                    nc.sync.dma_start(gsb[0:ow[1], 1:2], gv[li][128:128 + ow[1], :])
                    nc.sync.dma_start(bsb[0:ow[1], 1:2], bv[li][128:128 + ow[1], :])
                cntg = float(B * N * K)
                for ot in range(OT):
                    w_ = ow[ot]
                    nc.scalar.mul(mean[0:w_, ot:ot + 1], ar_out[0:w_, 2 * ot:2 * ot + 1], 1.0 / cntg)
                    nc.scalar.mul(e2[0:w_, ot:ot + 1], ar_out[0:w_, 2 * ot + 1:2 * ot + 2], 1.0 / cntg)
                    nc.vector.tensor_tensor(out=varp[0:w_, ot:ot + 1], in0=mean[0:w_, ot:ot + 1],
                                            in1=mean[0:w_, ot:ot + 1], op=Alu.mult)
                    nc.vector.scalar_tensor_tensor(out=varp[0:w_, ot:ot + 1],
                                                   in0=varp[0:w_, ot:ot + 1], scalar=-1.0,
                                                   in1=e2[0:w_, ot:ot + 1],
                                                   op0=Alu.mult, op1=Alu.add)
                    nc.vector.tensor_scalar_add(varp[0:w_, ot:ot + 1], varp[0:w_, ot:ot + 1], EPS)
                    nc.vector.reciprocal(rec[0:w_, ot:ot + 1], varp[0:w_, ot:ot + 1])
                    nc.scalar.sqrt(rsq[0:w_, ot:ot + 1], rec[0:w_, ot:ot + 1])
                    nc.vector.tensor_tensor(out=aco[0:w_, ot:ot + 1], in0=gsb[0:w_, ot:ot + 1],
                                            in1=rsq[0:w_, ot:ot + 1], op=Alu.mult)
                    # bias2 = b + a*(mu_u - mean) = b - a*(neg_mu + mean)
                    nc.vector.tensor_tensor(out=bi[0:w_, ot:ot + 1], in0=neg_mu[0:w_, ot:ot + 1],
                                            in1=mean[0:w_, ot:ot + 1], op=Alu.add)
                    nc.vector.tensor_tensor(out=bi[0:w_, ot:ot + 1], in0=bi[0:w_, ot:ot + 1],
                                            in1=aco[0:w_, ot:ot + 1], op=Alu.mult)
                    nc.vector.scalar_tensor_tensor(out=bi[0:w_, ot:ot + 1],
                                                   in0=bi[0:w_, ot:ot + 1], scalar=-1.0,
                                                   in1=bsb[0:w_, ot:ot + 1],
                                                   op0=Alu.mult, op1=Alu.add)
                    # z = a*(m' + v) + bias2 ; lrelu
                    if li < 3:
                        dst = hdst[li]
                    else:
                        dst = h[2][:, :] if ot == 0 else h[3][:, :]
                    dsl = dst if li == 3 else dst
                    zt = pp.tile([128, N], dt.float32, tag="zt")
                    nc.vector.tensor_tensor(out=zt[0:w_, :], in0=mm[0:w_, ot * N:(ot + 1) * N],
                                            in1=vv[0:w_, ot * N:(ot + 1) * N], op=Alu.add)
                    nc.scalar.activation(zt[0:w_, :], zt[0:w_, :], Act.Identity,
                                         bias=bi[0:w_, ot:ot + 1], scale=aco[0:w_, ot:ot + 1])
                    nc.vector.scalar_tensor_tensor(out=dsl[0:w_, :] if li == 3 else dst[0:w_, :],
                                                   in0=zt[0:w_, :], scalar=0.2,
                                                   in1=zt[0:w_, :], op0=Alu.mult, op1=Alu.max)
                    if li == 1:
                        nc.sync.dma_start(h[0][64:128, :], x2_sb)

        for i_ in range(4):
            nc.sync.dma_start(dbg_h[i_], h[i_])
        # ---------------- final conv + BN + lrelu ----------------
        with tc.tile_pool(name="pf", bufs=1) as pf, \
             tc.tile_pool(name="pfp", bufs=2, space="PSUM") as pfp:
            y_sb = []
            for ob in range(8):
                ytile = pf.tile([128, N], dt.float32, tag=f"y{ob}")
                y_sb.append(ytile)
            sm5 = pf.tile([128, 160], dt.float32, tag="sm5")
            sum_y = sm5[:, 0:8]
            mu5 = sm5[:, 8:16]
            nmu5 = sm5[:, 16:24]
            syc2 = sm5[:, 24:32]
            tE = sm5[:, 32:40]
            tF = sm5[:, 40:48]
            g5_sb = sm5[:, 48:56]
            b5_sb = sm5[:, 56:64]
            mean5 = sm5[:, 64:72]
            e25 = sm5[:, 72:80]
            var5 = sm5[:, 80:88]
            rec5 = sm5[:, 88:96]
            rsq5 = sm5[:, 96:104]
            a5 = sm5[:, 104:112]
            c5 = sm5[:, 112:120]
            ar5_in = sm5[:, 120:136]
            ar5_out = sm5[:, 136:152]
            for ob in range(8):
                for nch in range(NCH):
                    yp = pfp.tile([128, 512], dt.float32, tag="yp")
                    for cb in range(4):
                        nc.tensor.matmul(yp, lhsT=w5T_sb[:, cb * 1024 + ob * 128:
                                                         cb * 1024 + (ob + 1) * 128],
                                         rhs=h[cb][:, nch * 512:(nch + 1) * 512],
                                         start=(cb == 0), stop=(cb == 3))
                    nc.scalar.activation(y_sb[ob][:, nch * 512:(nch + 1) * 512], yp, Act.Copy)
                nc.vector.tensor_reduce(out=sum_y[:, ob:ob + 1], in_=y_sb[ob],
                                        op=Alu.add, axis=mybir.AxisListType.X)
                nc.scalar.mul(mu5[:, ob:ob + 1], sum_y[:, ob:ob + 1], 1.0 / N)
                nc.scalar.mul(nmu5[:, ob:ob + 1], sum_y[:, ob:ob + 1], -1.0 / N)
                yc = pf.tile([128, N], dt.float32, tag="yc")
                nc.scalar.activation(yc, y_sb[ob], Act.Identity,
                                     bias=nmu5[:, ob:ob + 1], scale=1.0)
                junk5 = pf.tile([128, N], dt.float32, tag="junk5")
                nc.vector.scalar_tensor_tensor(out=junk5, in0=yc, scalar=1.0, in1=yc,
                                               op0=Alu.mult, op1=Alu.mult,
                                               accum_out=syc2[:, ob:ob + 1])
                # t1 = sum_y ; t2 = syc2 + 2*mu5*(sum_y - N*mu5) + N*mu5^2
                #    = syc2 + 2*mu5*sum_y - N*mu5^2
                nc.vector.tensor_copy(ar5_in[:, 2 * ob:2 * ob + 1], sum_y[:, ob:ob + 1])
                nc.vector.tensor_tensor(out=tE[:, ob:ob + 1], in0=mu5[:, ob:ob + 1],
                                        in1=sum_y[:, ob:ob + 1], op=Alu.mult)
                nc.vector.scalar_tensor_tensor(out=tE[:, ob:ob + 1], in0=tE[:, ob:ob + 1],
                                               scalar=2.0, in1=syc2[:, ob:ob + 1],
                                               op0=Alu.mult, op1=Alu.add)
                nc.vector.tensor_tensor(out=tF[:, ob:ob + 1], in0=mu5[:, ob:ob + 1],
                                        in1=mu5[:, ob:ob + 1], op=Alu.mult)
                nc.vector.scalar_tensor_tensor(out=ar5_in[:, 2 * ob + 1:2 * ob + 2],
                                               in0=tF[:, ob:ob + 1], scalar=-float(N),
                                               in1=tE[:, ob:ob + 1],
                                               op0=Alu.mult, op1=Alu.add)
            with tc.tile_pool(name="dr5", bufs=1, space="DRAM") as dram5:
                ari5 = dram5.tile([128, 16], dt.float32)
                aro5 = dram5.tile([128, 16], dt.float32)
                nc.sync.dma_start(ari5[:], ar5_in)
                nc.gpsimd.collective_compute(
                    "AllReduce", Alu.add, replica_groups=[list(range(8))],
                    ins=[ari5.opt()], outs=[aro5.opt()])
                nc.sync.dma_start(ar5_out, aro5[:])
            for ob in range(8):
                nc.sync.dma_start(g5_sb[:, ob:ob + 1], g5_d[ob * 128:(ob + 1) * 128, :])
                nc.sync.dma_start(b5_sb[:, ob:ob + 1], b5_d[ob * 128:(ob + 1) * 128, :])
            cnt5 = float(B * N)
            for ob in range(8):
                nc.scalar.mul(mean5[:, ob:ob + 1], ar5_out[:, 2 * ob:2 * ob + 1], 1.0 / cnt5)
                nc.scalar.mul(e25[:, ob:ob + 1], ar5_out[:, 2 * ob + 1:2 * ob + 2], 1.0 / cnt5)
                nc.vector.tensor_tensor(out=var5[:, ob:ob + 1], in0=mean5[:, ob:ob + 1],
                                        in1=mean5[:, ob:ob + 1], op=Alu.mult)
                nc.vector.scalar_tensor_tensor(out=var5[:, ob:ob + 1], in0=var5[:, ob:ob + 1],
                                               scalar=-1.0, in1=e25[:, ob:ob + 1],
                                               op0=Alu.mult, op1=Alu.add)
                nc.vector.tensor_scalar_add(var5[:, ob:ob + 1], var5[:, ob:ob + 1], EPS)
                nc.vector.reciprocal(rec5[:, ob:ob + 1], var5[:, ob:ob + 1])
                nc.scalar.sqrt(rsq5[:, ob:ob + 1], rec5[:, ob:ob + 1])
                nc.vector.tensor_tensor(out=a5[:, ob:ob + 1], in0=g5_sb[:, ob:ob + 1],
                                        in1=rsq5[:, ob:ob + 1], op=Alu.mult)
                nc.vector.tensor_tensor(out=c5[:, ob:ob + 1], in0=mean5[:, ob:ob + 1],
                                        in1=a5[:, ob:ob + 1], op=Alu.mult)
                nc.vector.scalar_tensor_tensor(out=c5[:, ob:ob + 1], in0=c5[:, ob:ob + 1],
                                               scalar=-1.0, in1=b5_sb[:, ob:ob + 1],
                                               op0=Alu.mult, op1=Alu.add)
                z5 = pf.tile([128, N], dt.float32, tag="z5")
                nc.scalar.activation(z5, y_sb[ob], Act.Identity,
                                     bias=c5[:, ob:ob + 1], scale=a5[:, ob:ob + 1])
                o5 = pf.tile([128, N], dt.float32, tag="o5")
                nc.vector.scalar_tensor_tensor(out=o5, in0=z5, scalar=0.2, in1=z5,
                                               op0=Alu.mult, op1=Alu.max)
                nc.sync.dma_start(out_d[ob * 128:(ob + 1) * 128, :], o5)

    nc.compile()
    return nc


def _get_compiled():
    if "nc" not in _CACHE:
        _CACHE["nc"] = _build()
    return _CACHE["nc"]


def _make_in_maps(inputs):
    x = np.ascontiguousarray(np.asarray(inputs["x"], dtype=np.float32))
    shared = {}
    Cs = [3, 64, 64, 128]
    for li in range(4):
        w = np.asarray(inputs[f"w{li+1}"], dtype=np.float32)
        C = Cs[li]
        shared[f"waT{li}"] = np.ascontiguousarray(w[:, :C].T)
        shared[f"wbmaT{li}"] = np.ascontiguousarray((w[:, C:] - w[:, :C]).T)
        shared[f"g{li}"] = np.ascontiguousarray(
            np.asarray(inputs[f"g{li+1}"], np.float32).reshape(-1, 1))
        shared[f"b{li}"] = np.ascontiguousarray(
            np.asarray(inputs[f"b{li+1}"], np.float32).reshape(-1, 1))
    shared["w5T"] = np.ascontiguousarray(np.asarray(inputs["w5"], np.float32).T)
    shared["g5"] = np.ascontiguousarray(np.asarray(inputs["g5"], np.float32).reshape(-1, 1))
    shared["b5"] = np.ascontiguousarray(np.asarray(inputs["b5"], np.float32).reshape(-1, 1))

    return [dict(shared, x0=np.ascontiguousarray(x[i])) for i in range(B)]


def kernel(**inputs):
    from concourse.bass_utils import run_bass_kernel_spmd

    nc = _get_compiled()
    in_maps = _make_in_maps(inputs)
    res = run_bass_kernel_spmd(nc, in_maps, core_ids=list(range(8)))
    out = np.stack([res.results[i]["out"] for i in range(B)]).astype(np.float32)
    return out



# revision 10
# speedup vs baseline: 1.2364x; 1.2364x over previous
"""DGCNN forward kernel for Trainium2 (8 NeuronCores, batch-parallel).

Strategy (per core = one sample of the batch):
  - kNN scores S[n,c] = 2<x_n,x_c> - ||x_c||^2 computed directly on PE via an
    augmented matmul  S = [x;1]^T @ [2x;-xx]  (row and column orientations are
    bit-identical, so thresholds are consistent).
  - top-20 per row via 3 rounds of DVE max8/max_index + match_replace.
  - EdgeConv decomposition: y[o,n,j] = u[o, idx[n,j]] + v[o,n] with
    u = wa@x (neighbor part), v = (wb-wa)@x (center part). max_j then needs a
    gather-max over u only; gathers run on GPSIMD (ap_gather), segmented max on
    DVE pools.
  - BN statistics (training-mode, over the whole batch) from the SAME gathered
    u' tiles: s[o,n] = sum_j u'[idx[n,j]] via DVE add-reduce, sum of u'^2 via
    ACT Square+accum; u' mean-centered to avoid fp32 catastrophic cancellation;
    global sums via a tiny AllReduce (syncBN) per layer.
  - Final 1x1 conv + BN + leaky relu, stats again via AllReduce.
"""

import numpy as np

B, C0, N = 8, 3, 2048
K = 20
EPS = 1e-5
LAYERS = [(3, 64), (64, 64), (64, 128), (128, 256)]  # (C_in, O)
NT = N // 128          # 16 row tiles
NCH = N // 512         # 4 matmul free-dim chunks
NEG = -1.0e38

_CACHE = {}


def _build():
    import concourse.bass as bass
    import concourse.mybir as mybir
    from concourse import bacc
    from concourse.tile import TileContext

    dt = mybir.dt
    Alu = mybir.AluOpType
    Act = mybir.ActivationFunctionType

    nc = bacc.Bacc("TRN2", target_bir_lowering=False, debug=False,
                   enable_asserts=False, num_devices=8)

    # ---------------- DRAM I/O ----------------
    x_in = nc.dram_tensor("x0", [C0, N], dt.float32, kind="ExternalInput").ap()
    waT, wbmaT, gv, bv = {}, {}, {}, {}
    for li, (C, O) in enumerate(LAYERS):
        waT[li] = nc.dram_tensor(f"waT{li}", [C, O], dt.float32, kind="ExternalInput").ap()
        wbmaT[li] = nc.dram_tensor(f"wbmaT{li}", [C, O], dt.float32, kind="ExternalInput").ap()
        gv[li] = nc.dram_tensor(f"g{li}", [O, 1], dt.float32, kind="ExternalInput").ap()
        bv[li] = nc.dram_tensor(f"b{li}", [O, 1], dt.float32, kind="ExternalInput").ap()
    w5T_d = nc.dram_tensor("w5T", [512, 1024], dt.float32, kind="ExternalInput").ap()
    g5_d = nc.dram_tensor("g5", [1024, 1], dt.float32, kind="ExternalInput").ap()
    b5_d = nc.dram_tensor("b5", [1024, 1], dt.float32, kind="ExternalInput").ap()
    out_d = nc.dram_tensor("out", [1024, N], dt.float32, kind="ExternalOutput").ap()

    def sb(name, shape, dtype=dt.float32):
        return nc.alloc_sbuf_tensor(name, list(shape), dtype).ap()

    with TileContext(nc) as tc:
        # ---------------- persistent SBUF ----------------
        h = [sb("h0", [128, N]), sb("h1", [128, N]),
             sb("h2", [128, N]), sb("h3", [128, N])]
        ones_row = sb("ones_row", [1, N])
        nc.vector.memset(ones_row, 1.0)
        # identity for PE transpose: ident[p, f] = (f - p == 0)
        ident = sb("ident", [128, 128])
        iota_fp = sb("iota_fp", [128, 128], dt.int32)
        nc.gpsimd.iota(iota_fp, pattern=[[1, 128]], base=0, channel_multiplier=-1)
        nc.vector.tensor_scalar(out=ident, in0=iota_fp, scalar1=0, scalar2=None,
                                op0=Alu.is_equal)

        # x of current layer lives in h-slices; layer0 input loaded separately
        x0_sb = sb("x0_sb", [C0, N])
        nc.sync.dma_start(x0_sb, x_in)

        w5T_sb = sb("w5T_sb", [128, 4 * 1024])   # 4 c-blocks side by side
        for cb in range(4):
            nc.sync.dma_start(w5T_sb[:, cb * 1024:(cb + 1) * 1024],
                              w5T_d[cb * 128:(cb + 1) * 128, :])

        # x2 needs its own base-partition-0 tensor (matmul operands must share base)
        x2_sb = sb("x2_sb", [64, N])
        x_of = {0: x0_sb[:, :], 1: h[0][0:64, :], 2: x2_sb[:, :], 3: h[1][:, :]}
        hdst = {0: h[0][0:64, :], 1: x2_sb[:, :], 2: h[1][:, :],
                3: None}  # layer3 output 256ch -> h2,h3

        for li, (C, O) in enumerate(LAYERS):
            xc = x_of[li]
            OT = (O + 127) // 128            # o-tiles
            ow = [min(128, O - ot * 128) for ot in range(OT)]

            with tc.tile_pool(name=f"pp{li}", bufs=1) as pp:
              with tc.tile_pool(name=f"pt{li}", bufs=3, space="PSUM") as psT:
                # ---- augmented operands ----
                b2x = pp.tile([C, N], dt.float32, tag="b2x")
                nc.scalar.mul(b2x, xc, 2.0)
                xsq = pp.tile([C, N], dt.float32, tag="zt")
                nc.vector.tensor_tensor(out=xsq, in0=xc, in1=xc, op=Alu.mult)
                ones_col = pp.tile([C, 1], dt.float32, tag="ones_col")
                nc.vector.memset(ones_col, 1.0)
                bnxx = pp.tile([1, N], dt.float32, tag="bnxx")
                for nch in range(NCH):
                    xxp = psT.tile([1, 512], dt.float32, tag="pt512")
                    nc.tensor.matmul(xxp, lhsT=ones_col, rhs=xsq[:, nch * 512:(nch + 1) * 512],
                                     start=True, stop=True)
                    nc.scalar.mul(bnxx[:, nch * 512:(nch + 1) * 512], xxp, -1.0)

                # ---- u, v, centering ----
                waT_sb = pp.tile([C, O], dt.float32, tag="waT_sb")
                nc.sync.dma_start(waT_sb, waT[li])
                wbmaT_sb = pp.tile([C, O], dt.float32, tag="wbmaT_sb")
                nc.sync.dma_start(wbmaT_sb, wbmaT[li])
                up = pp.tile([128, OT * N], dt.float32, tag="up")       # u' (centered), o-tiles side by side
                vv = pp.tile([128, OT * N], dt.float32, tag="vv")
                sm = pp.tile([128, 96], dt.float32, tag="sm")
                neg_mu = sm[:, 0:2]
                sum_v = sm[:, 2:4]
                sum_u = sm[:, 4:6]
                sum_q = sm[:, 6:8]
                sum_s = sm[:, 8:10]
                svs = sm[:, 10:12]
                sv2 = sm[:, 12:14]
                vbar = sm[:, 14:16]
                beta = sm[:, 16:18]
                t1a = sm[:, 18:20]
                S1p = sm[:, 20:22]
                tA = sm[:, 22:24]
                tBt = sm[:, 24:26]
                S2p = sm[:, 26:28]
                tC = sm[:, 28:30]
                tD = sm[:, 30:32]
                mean = sm[:, 32:34]
                e2 = sm[:, 34:36]
                varp = sm[:, 36:38]
                rec = sm[:, 38:40]
                rsq = sm[:, 40:42]
                aco = sm[:, 42:44]
                bi = sm[:, 44:46]
                gsb = sm[:, 46:48]
                bsb = sm[:, 48:50]
                ar_in = sm[:, 50:54]
                ar_out = sm[:, 54:58]
                for ot in range(OT):
                    w_ = ow[ot]
                    for nch in range(NCH):
                        upp = psT.tile([128, 512], dt.float32, tag="pt512")
                        nc.tensor.matmul(upp[0:w_, :], lhsT=waT_sb[:, ot * 128:ot * 128 + w_],
                                         rhs=xc[:, nch * 512:(nch + 1) * 512], start=True, stop=True)
                        nc.scalar.activation(up[0:w_, ot * N + nch * 512: ot * N + (nch + 1) * 512],
                                             upp[0:w_, :], Act.Copy)
                        vpp = psT.tile([128, 512], dt.float32, tag="pt512")
                        nc.tensor.matmul(vpp[0:w_, :], lhsT=wbmaT_sb[:, ot * 128:ot * 128 + w_],
                                         rhs=xc[:, nch * 512:(nch + 1) * 512], start=True, stop=True)
                        nc.scalar.activation(vv[0:w_, ot * N + nch * 512: ot * N + (nch + 1) * 512],
                                             vpp[0:w_, :], Act.Copy)
                    nc.vector.tensor_reduce(out=sum_u[0:w_, ot:ot + 1],
                                            in_=up[0:w_, ot * N:(ot + 1) * N],
                                            op=Alu.add, axis=mybir.AxisListType.X)
                    nc.vector.tensor_reduce(out=sum_v[0:w_, ot:ot + 1],
                                            in_=vv[0:w_, ot * N:(ot + 1) * N],
                                            op=Alu.add, axis=mybir.AxisListType.X)
                    nc.scalar.mul(neg_mu[0:w_, ot:ot + 1], sum_u[0:w_, ot:ot + 1], -1.0 / N)
                    # center u in place
                    nc.scalar.activation(up[0:w_, ot * N:(ot + 1) * N],
                                         up[0:w_, ot * N:(ot + 1) * N], Act.Identity,
                                         bias=neg_mu[0:w_, ot:ot + 1], scale=1.0)

                # ---- phase A: scores + topk per row tile ----
                list16 = pp.tile([16, NT * 128], dt.int16, tag="list16")
                list8 = pp.tile([16, NT * 64], dt.int16, tag="list8")
                with tc.tile_pool(name=f"pa{li}", bufs=2) as pa:
                    for rt in range(NT):
                        Ssb = pa.tile([128, N], dt.float32, tag="Ssb")
                        for nch in range(NCH):
                            Spc = psT.tile([128, 512], dt.float32, tag="pt512")
                            nc.tensor.matmul(Spc,
                                             lhsT=xc[:, rt * 128:(rt + 1) * 128],
                                             rhs=b2x[:, nch * 512:(nch + 1) * 512],
                                             start=True, stop=False)
                            nc.tensor.matmul(Spc,
                                             lhsT=ones_row[:, rt * 128:(rt + 1) * 128],
                                             rhs=bnxx[:, nch * 512:(nch + 1) * 512],
                                             start=False, stop=True)
                            nc.scalar.activation(Ssb[:, nch * 512:(nch + 1) * 512], Spc, Act.Copy)
                        V = pa.tile([128, 24], dt.float32, tag="V")
                        I = pa.tile([128, 24], dt.uint16, tag="I")
                        Sw = pa.tile([128, N], dt.float32, tag="Ssb")   # share slots with Ssb
                        nc.vector.max(out=V[:, 0:8], in_=Ssb)
                        nc.vector.max_index(out=I[:, 0:8], in_max=V[:, 0:8], in_values=Ssb)
                        nc.vector.match_replace(out=Sw, in_to_replace=V[:, 0:8],
                                                in_values=Ssb, imm_value=NEG)
                        nc.vector.max(out=V[:, 8:16], in_=Sw)
                        nc.vector.max_index(out=I[:, 8:16], in_max=V[:, 8:16], in_values=Sw)
                        nc.vector.match_replace(out=Sw, in_to_replace=V[:, 8:16],
                                                in_values=Sw, imm_value=NEG)
                        nc.vector.max(out=V[:, 16:24], in_=Sw)
                        nc.vector.max_index(out=I[:, 16:24], in_max=V[:, 16:24], in_values=Sw)
                        # index lists
                        If = pa.tile([128, 24], dt.float32, tag="If")
                        nc.vector.tensor_copy(If, I)
                        tpa = psT.tile([128, 128], dt.float32, tag="pt512")
                        nc.tensor.transpose(tpa[0:16, :], If[:, 0:16], ident)
                        nc.vector.tensor_copy(list16[:, rt * 128:(rt + 1) * 128], tpa[0:16, :])
                        tpb = psT.tile([128, 128], dt.float32, tag="pt512")
                        nc.tensor.transpose(tpb[0:8, :], If[:, 16:24], ident)
                        st8 = pa.tile([8, 128], dt.int16, tag="st8")
                        for bb in range(2):
                            nc.vector.tensor_copy(st8[:, bb * 64:(bb + 1) * 64],
                                                  tpb[0:8, bb::2])
                            nc.sync.dma_start(
                                list8[bb * 8:(bb + 1) * 8, rt * 64:(rt + 1) * 64],
                                st8[:, bb * 64:(bb + 1) * 64])

                # replicate gather lists into all channel groups
                CH = 128 if O >= 128 else 64
                lg1 = pp.tile([CH, NT * 128], dt.int16, tag="lg1")
                lg2 = pp.tile([CH, NT * 64], dt.int16, tag="lg2")
                for gch in range(CH // 16):
                    nc.sync.dma_start(lg1[gch * 16:(gch + 1) * 16, :], list16)
                    nc.sync.dma_start(lg2[gch * 16:(gch + 1) * 16, :], list8)

                # ---- gather over u': max (for output), sum (s), square-sum (q) ----
                mm = pp.tile([128, OT * N], dt.float32, tag="mm")      # m' per o-tile
                s_sb = pp.tile([128, OT * N], dt.float32, tag="s_sb")  # s[o,n] = sum_j u'[idx]
                qac = pp.tile([128, OT * 64], dt.float32, tag="qac")   # per-chunk sq-sum accums
                with tc.tile_pool(name=f"pg{li}", bufs=2) as pg:
                    for ot in range(OT):
                        w_ = ow[ot]
                        wch = ((w_ + 15) // 16) * 16
                        usrc = up[0:wch, ot * N:(ot + 1) * N]
                        CKR = 64                        # rows per gather chunk
                        for ck in range(N // CKR):
                            g1 = pg.tile([CH, CKR * 16], dt.float32, tag="g1")
                            nc.gpsimd.ap_gather(
                                g1[0:wch, :], usrc,
                                lg1[0:wch, ck * CKR: (ck + 1) * CKR],
                                channels=wch, num_elems=N, d=1, num_idxs=CKR * 16)
                            nc.vector.tensor_reduce(
                                out=mm[0:w_, ot * N + ck * CKR: ot * N + (ck + 1) * CKR],
                                in_=g1[0:w_, :].rearrange("p (n k) -> p n k", k=16),
                                op=Alu.max, axis=mybir.AxisListType.X)
                            g2 = pg.tile([CH, CKR * 8], dt.float32, tag="g2")
                            nc.gpsimd.ap_gather(
                                g2[0:wch, :], usrc,
                                lg2[0:wch, ck * (CKR // 2): (ck + 1) * (CKR // 2)],
                                channels=wch, num_elems=N, d=1, num_idxs=CKR * 8)
                            m2 = pg.tile([128, CKR], dt.float32, tag="m2")
                            nc.vector.tensor_reduce(
                                out=m2[0:w_, :],
                                in_=g2[0:w_, :].rearrange("p (n k) -> p n k", k=8)[:, :, 0:4],
                                op=Alu.max, axis=mybir.AxisListType.X)
                            nc.vector.tensor_tensor(
                                out=mm[0:w_, ot * N + ck * CKR: ot * N + (ck + 1) * CKR],
                                in0=mm[0:w_, ot * N + ck * CKR: ot * N + (ck + 1) * CKR],
                                in1=m2[0:w_, :], op=Alu.max)
                            # s: sum over the 20 neighbors (16 from g1 + first 4 of g2)
                            s1 = pg.tile([128, CKR], dt.float32, tag="s1")
                            nc.vector.tensor_reduce(
                                out=s1[0:w_, :],
                                in_=g1[0:w_, :].rearrange("p (n k) -> p n k", k=16),
                                op=Alu.add, axis=mybir.AxisListType.X)
                            s2 = pg.tile([128, CKR], dt.float32, tag="s2")
                            nc.vector.tensor_reduce(
                                out=s2[0:w_, :],
                                in_=g2[0:w_, :].rearrange("p (n k) -> p n k", k=8)[:, :, 0:4],
                                op=Alu.add, axis=mybir.AxisListType.X)
                            nc.vector.tensor_tensor(
                                out=s_sb[0:w_, ot * N + ck * CKR: ot * N + (ck + 1) * CKR],
                                in0=s1[0:w_, :], in1=s2[0:w_, :], op=Alu.add)
                            # q: global sum of u'^2 over same neighbors (ACT square+accum)
                            qj = pg.tile([CH, CKR * 16], dt.float32, tag="qj")
                            nc.scalar.activation(
                                qj[0:w_, :], g1[0:w_, :], Act.Square,
                                accum_out=qac[0:w_, ot * 64 + 2 * ck: ot * 64 + 2 * ck + 1])
                            qj2 = pg.tile([CH, CKR * 4], dt.float32, tag="qj2")
                            nc.scalar.activation(
                                qj2[0:w_, :].rearrange("p (n k) -> p n k", k=4),
                                g2[0:w_, :].rearrange("p (n k) -> p n k", k=8)[:, :, 0:4],
                                Act.Square,
                                accum_out=qac[0:w_, ot * 64 + 2 * ck + 1: ot * 64 + 2 * ck + 2])
                for ot in range(OT):
                    w_ = ow[ot]
                    nc.vector.tensor_reduce(out=sum_q[0:w_, ot:ot + 1],
                                            in_=qac[0:w_, ot * 64:(ot + 1) * 64],
                                            op=Alu.add, axis=mybir.AxisListType.X)

                # ---- per-core stat terms + AllReduce ----
                for ot in range(OT):
                    w_ = ow[ot]
                    ssl = s_sb[0:w_, ot * N:(ot + 1) * N]
                    vsl = vv[0:w_, ot * N:(ot + 1) * N]
                    nc.vector.tensor_reduce(out=sum_s[0:w_, ot:ot + 1], in_=ssl,
                                            op=Alu.add, axis=mybir.AxisListType.X)
                    junk = pp.tile([128, N], dt.float32, tag="zt")
                    nc.vector.scalar_tensor_tensor(out=junk[0:w_, :], in0=ssl, scalar=1.0,
                                                   in1=vsl, op0=Alu.mult, op1=Alu.mult,
                                                   accum_out=svs[0:w_, ot:ot + 1])
                    nc.vector.scalar_tensor_tensor(out=junk[0:w_, :], in0=vsl, scalar=1.0,
                                                   in1=vsl, op0=Alu.mult, op1=Alu.mult,
                                                   accum_out=sv2[0:w_, ot:ot + 1])
                    # small [w_,1] algebra on DVE/ACT:
                    nc.scalar.mul(vbar[0:w_, ot:ot + 1], sum_v[0:w_, ot:ot + 1], 1.0 / N)
                    nc.vector.scalar_tensor_tensor(out=beta[0:w_, ot:ot + 1],
                                                   in0=neg_mu[0:w_, ot:ot + 1], scalar=-1.0,
                                                   in1=vbar[0:w_, ot:ot + 1],
                                                   op0=Alu.mult, op1=Alu.add)
                    # S1' = sum_s + K*(sum_v - N*vbar);  sum_v - N*vbar == 0 exactly? keep it:
                    nc.vector.scalar_tensor_tensor(out=t1a[0:w_, ot:ot + 1],
                                                   in0=vbar[0:w_, ot:ot + 1], scalar=-float(N),
                                                   in1=sum_v[0:w_, ot:ot + 1],
                                                   op0=Alu.mult, op1=Alu.add)  # sum_v - N*vbar
                    nc.vector.scalar_tensor_tensor(out=S1p[0:w_, ot:ot + 1],
                                                   in0=t1a[0:w_, ot:ot + 1], scalar=float(K),
                                                   in1=sum_s[0:w_, ot:ot + 1],
                                                   op0=Alu.mult, op1=Alu.add)
                    # S2' = sum_q + 2*(svs - vbar*sum_s) + K*(sv2 - N*vbar^2)
                    nc.vector.tensor_tensor(out=tA[0:w_, ot:ot + 1], in0=vbar[0:w_, ot:ot + 1],
                                            in1=sum_s[0:w_, ot:ot + 1], op=Alu.mult)
                    nc.vector.scalar_tensor_tensor(out=tA[0:w_, ot:ot + 1],
                                                   in0=tA[0:w_, ot:ot + 1], scalar=-1.0,
                                                   in1=svs[0:w_, ot:ot + 1],
                                                   op0=Alu.mult, op1=Alu.add)  # svs - vbar*sum_s
                    nc.vector.tensor_tensor(out=tBt[0:w_, ot:ot + 1], in0=vbar[0:w_, ot:ot + 1],
                                            in1=vbar[0:w_, ot:ot + 1], op=Alu.mult)
                    nc.vector.scalar_tensor_tensor(out=tBt[0:w_, ot:ot + 1],
                                                   in0=tBt[0:w_, ot:ot + 1], scalar=-float(N),
                                                   in1=sv2[0:w_, ot:ot + 1],
                                                   op0=Alu.mult, op1=Alu.add)  # sv2 - N*vbar^2
                    nc.vector.scalar_tensor_tensor(out=S2p[0:w_, ot:ot + 1],
                                                   in0=tA[0:w_, ot:ot + 1], scalar=2.0,
                                                   in1=sum_q[0:w_, ot:ot + 1],
                                                   op0=Alu.mult, op1=Alu.add)
                    nc.vector.scalar_tensor_tensor(out=S2p[0:w_, ot:ot + 1],
                                                   in0=tBt[0:w_, ot:ot + 1], scalar=float(K),
                                                   in1=S2p[0:w_, ot:ot + 1],
                                                   op0=Alu.mult, op1=Alu.add)
                    # t1 = S1' + cnt*beta ; t2 = S2' + 2*beta*S1' + cnt*beta^2
                    cntl = float(N * K)
                    nc.vector.scalar_tensor_tensor(out=ar_in[0:w_, 2 * ot:2 * ot + 1],
                                                   in0=beta[0:w_, ot:ot + 1], scalar=cntl,
                                                   in1=S1p[0:w_, ot:ot + 1],
                                                   op0=Alu.mult, op1=Alu.add)
                    nc.vector.tensor_tensor(out=tC[0:w_, ot:ot + 1], in0=beta[0:w_, ot:ot + 1],
                                            in1=S1p[0:w_, ot:ot + 1], op=Alu.mult)
                    nc.vector.scalar_tensor_tensor(out=tC[0:w_, ot:ot + 1],
                                                   in0=tC[0:w_, ot:ot + 1], scalar=2.0,
                                                   in1=S2p[0:w_, ot:ot + 1],
                                                   op0=Alu.mult, op1=Alu.add)
                    nc.vector.tensor_tensor(out=tD[0:w_, ot:ot + 1], in0=beta[0:w_, ot:ot + 1],
                                            in1=beta[0:w_, ot:ot + 1], op=Alu.mult)
                    nc.vector.scalar_tensor_tensor(out=ar_in[0:w_, 2 * ot + 1:2 * ot + 2],
                                                   in0=tD[0:w_, ot:ot + 1], scalar=cntl,
                                                   in1=tC[0:w_, ot:ot + 1],
                                                   op0=Alu.mult, op1=Alu.add)

                with tc.tile_pool(name=f"dr{li}", bufs=1, space="DRAM") as dram:
                    ari = dram.tile([128, 2 * OT], dt.float32)
                    aro = dram.tile([128, 2 * OT], dt.float32)
                    nc.sync.dma_start(ari[:], ar_in[:, 0:2 * OT])
                    nc.gpsimd.collective_compute(
                        "AllReduce", Alu.add, replica_groups=[list(range(8))],
                        ins=[ari.opt()], outs=[aro.opt()])
                    nc.sync.dma_start(ar_out[:, 0:2 * OT], aro[:])

                # post-AR: mean/var/scale/bias + activation
                nc.sync.dma_start(gsb[0:ow[0], 0:1], gv[li][0:ow[0], :])
                nc.sync.dma_start(bsb[0:ow[0], 0:1], bv[li][0:ow[0], :])
                if OT > 1:
                    nc.sync.dma_start(gsb[0:ow[1], 1:2], gv[li][128:128 + ow[1], :])
                    nc.sync.dma_start(bsb[0:ow[1], 1:2], bv[li][128:128 + ow[1], :])
                cntg = float(B * N * K)
                for ot in range(OT):
                    w_ = ow[ot]
                    nc.scalar.mul(mean[0:w_, ot:ot + 1], ar_out[0:w_, 2 * ot:2 * ot + 1], 1.0 / cntg)
                    nc.scalar.mul(e2[0:w_, ot:ot + 1], ar_out[0:w_, 2 * ot + 1:2 * ot + 2], 1.0 / cntg)
                    nc.vector.tensor_tensor(out=varp[0:w_, ot:ot + 1], in0=mean[0:w_, ot:ot + 1],
                                            in1=mean[0:w_, ot:ot + 1], op=Alu.mult)
                    nc.vector.scalar_tensor_tensor(out=varp[0:w_, ot:ot + 1],
                                                   in0=varp[0:w_, ot:ot + 1], scalar=-1.0,
                                                   in1=e2[0:w_, ot:ot + 1],
                                                   op0=Alu.mult, op1=Alu.add)
                    nc.vector.tensor_scalar_add(varp[0:w_, ot:ot + 1], varp[0:w_, ot:ot + 1], EPS)
                    nc.vector.reciprocal(rec[0:w_, ot:ot + 1], varp[0:w_, ot:ot + 1])
                    nc.scalar.sqrt(rsq[0:w_, ot:ot + 1], rec[0:w_, ot:ot + 1])
                    nc.vector.tensor_tensor(out=aco[0:w_, ot:ot + 1], in0=gsb[0:w_, ot:ot + 1],
                                            in1=rsq[0:w_, ot:ot + 1], op=Alu.mult)
                    # bias2 = b + a*(mu_u - mean) = b - a*(neg_mu + mean)
                    nc.vector.tensor_tensor(out=bi[0:w_, ot:ot + 1], in0=neg_mu[0:w_, ot:ot + 1],
                                            in1=mean[0:w_, ot:ot + 1], op=Alu.add)
                    nc.vector.tensor_tensor(out=bi[0:w_, ot:ot + 1], in0=bi[0:w_, ot:ot + 1],
                                            in1=aco[0:w_, ot:ot + 1], op=Alu.mult)
                    nc.vector.scalar_tensor_tensor(out=bi[0:w_, ot:ot + 1],
                                                   in0=bi[0:w_, ot:ot + 1], scalar=-1.0,
                                                   in1=bsb[0:w_, ot:ot + 1],
                                                   op0=Alu.mult, op1=Alu.add)
                    # z = a*(m' + v) + bias2 ; lrelu
                    if li < 3:
                        dst = hdst[li]
                    else:
                        dst = h[2][:, :] if ot == 0 else h[3][:, :]
                    dsl = dst if li == 3 else dst
                    zt = pp.tile([128, N], dt.float32, tag="zt")
                    nc.vector.tensor_tensor(out=zt[0:w_, :], in0=mm[0:w_, ot * N:(ot + 1) * N],
                                            in1=vv[0:w_, ot * N:(ot + 1) * N], op=Alu.add)
                    nc.scalar.activation(zt[0:w_, :], zt[0:w_, :], Act.Identity,
                                         bias=bi[0:w_, ot:ot + 1], scale=aco[0:w_, ot:ot + 1])
                    nc.vector.scalar_tensor_tensor(out=dsl[0:w_, :] if li == 3 else dst[0:w_, :],
                                                   in0=zt[0:w_, :], scalar=0.2,
                                                   in1=zt[0:w_, :], op0=Alu.mult, op1=Alu.max)
                    if li == 1:
                        nc.sync.dma_start(h[0][64:128, :], x2_sb)

        # ---------------- final conv + BN + lrelu ----------------
        with tc.tile_pool(name="pf", bufs=1) as pf, \
             tc.tile_pool(name="pfp", bufs=2, space="PSUM") as pfp:
            y_sb = []
            for ob in range(8):
                ytile = pf.tile([128, N], dt.float32, tag=f"y{ob}")
                y_sb.append(ytile)
            sm5 = pf.tile([128, 160], dt.float32, tag="sm5")
            sum_y = sm5[:, 0:8]
            mu5 = sm5[:, 8:16]
            nmu5 = sm5[:, 16:24]
            syc2 = sm5[:, 24:32]
            tE = sm5[:, 32:40]
            tF = sm5[:, 40:48]
            g5_sb = sm5[:, 48:56]
            b5_sb = sm5[:, 56:64]
            mean5 = sm5[:, 64:72]
            e25 = sm5[:, 72:80]
            var5 = sm5[:, 80:88]
            rec5 = sm5[:, 88:96]
            rsq5 = sm5[:, 96:104]
            a5 = sm5[:, 104:112]
            c5 = sm5[:, 112:120]
            ar5_in = sm5[:, 120:136]
            ar5_out = sm5[:, 136:152]
            for ob in range(8):
                for nch in range(NCH):
                    yp = pfp.tile([128, 512], dt.float32, tag="yp")
                    for cb in range(4):
                        nc.tensor.matmul(yp, lhsT=w5T_sb[:, cb * 1024 + ob * 128:
                                                         cb * 1024 + (ob + 1) * 128],
                                         rhs=h[cb][:, nch * 512:(nch + 1) * 512],
                                         start=(cb == 0), stop=(cb == 3))
                    nc.scalar.activation(y_sb[ob][:, nch * 512:(nch + 1) * 512], yp, Act.Copy)
                nc.vector.tensor_reduce(out=sum_y[:, ob:ob + 1], in_=y_sb[ob],
                                        op=Alu.add, axis=mybir.AxisListType.X)
                nc.scalar.mul(mu5[:, ob:ob + 1], sum_y[:, ob:ob + 1], 1.0 / N)
                nc.scalar.mul(nmu5[:, ob:ob + 1], sum_y[:, ob:ob + 1], -1.0 / N)
                yc = pf.tile([128, N], dt.float32, tag="yc")
                nc.scalar.activation(yc, y_sb[ob], Act.Identity,
                                     bias=nmu5[:, ob:ob + 1], scale=1.0)
                junk5 = pf.tile([128, N], dt.float32, tag="junk5")
                nc.vector.scalar_tensor_tensor(out=junk5, in0=yc, scalar=1.0, in1=yc,
                                               op0=Alu.mult, op1=Alu.mult,
                                               accum_out=syc2[:, ob:ob + 1])
                # t1 = sum_y ; t2 = syc2 + 2*mu5*(sum_y - N*mu5) + N*mu5^2
                #    = syc2 + 2*mu5*sum_y - N*mu5^2
                nc.vector.tensor_copy(ar5_in[:, 2 * ob:2 * ob + 1], sum_y[:, ob:ob + 1])
                nc.vector.tensor_tensor(out=tE[:, ob:ob + 1], in0=mu5[:, ob:ob + 1],
                                        in1=sum_y[:, ob:ob + 1], op=Alu.mult)
                nc.vector.scalar_tensor_tensor(out=tE[:, ob:ob + 1], in0=tE[:, ob:ob + 1],
                                               scalar=2.0, in1=syc2[:, ob:ob + 1],
                                               op0=Alu.mult, op1=Alu.add)
                nc.vector.tensor_tensor(out=tF[:, ob:ob + 1], in0=mu5[:, ob:ob + 1],
                                        in1=mu5[:, ob:ob + 1], op=Alu.mult)
                nc.vector.scalar_tensor_tensor(out=ar5_in[:, 2 * ob + 1:2 * ob + 2],
                                               in0=tF[:, ob:ob + 1], scalar=-float(N),
                                               in1=tE[:, ob:ob + 1],
                                               op0=Alu.mult, op1=Alu.add)
            with tc.tile_pool(name="dr5", bufs=1, space="DRAM") as dram5:
                ari5 = dram5.tile([128, 16], dt.float32)
                aro5 = dram5.tile([128, 16], dt.float32)
                nc.sync.dma_start(ari5[:], ar5_in)
                nc.gpsimd.collective_compute(
                    "AllReduce", Alu.add, replica_groups=[list(range(8))],
                    ins=[ari5.opt()], outs=[aro5.opt()])
                nc.sync.dma_start(ar5_out, aro5[:])
            for ob in range(8):
                nc.sync.dma_start(g5_sb[:, ob:ob + 1], g5_d[ob * 128:(ob + 1) * 128, :])
                nc.sync.dma_start(b5_sb[:, ob:ob + 1], b5_d[ob * 128:(ob + 1) * 128, :])
            cnt5 = float(B * N)
            for ob in range(8):
                nc.scalar.mul(mean5[:, ob:ob + 1], ar5_out[:, 2 * ob:2 * ob + 1], 1.0 / cnt5)
                nc.scalar.mul(e25[:, ob:ob + 1], ar5_out[:, 2 * ob + 1:2 * ob + 2], 1.0 / cnt5)
                nc.vector.tensor_tensor(out=var5[:, ob:ob + 1], in0=mean5[:, ob:ob + 1],
                                        in1=mean5[:, ob:ob + 1], op=Alu.mult)
                nc.vector.scalar_tensor_tensor(out=var5[:, ob:ob + 1], in0=var5[:, ob:ob + 1],
                                               scalar=-1.0, in1=e25[:, ob:ob + 1],
                                               op0=Alu.mult, op1=Alu.add)
                nc.vector.tensor_scalar_add(var5[:, ob:ob + 1], var5[:, ob:ob + 1], EPS)
                nc.vector.reciprocal(rec5[:, ob:ob + 1], var5[:, ob:ob + 1])
                nc.scalar.sqrt(rsq5[:, ob:ob + 1], rec5[:, ob:ob + 1])
                nc.vector.tensor_tensor(out=a5[:, ob:ob + 1], in0=g5_sb[:, ob:ob + 1],
                                        in1=rsq5[:, ob:ob + 1], op=Alu.mult)
                nc.vector.tensor_tensor(out=c5[:, ob:ob + 1], in0=mean5[:, ob:ob + 1],
                                        in1=a5[:, ob:ob + 1], op=Alu.mult)
                nc.vector.scalar_tensor_tensor(out=c5[:, ob:ob + 1], in0=c5[:, ob:ob + 1],
                                               scalar=-1.0, in1=b5_sb[:, ob:ob + 1],
                                               op0=Alu.mult, op1=Alu.add)
                z5 = pf.tile([128, N], dt.float32, tag="z5")
                nc.scalar.activation(z5, y_sb[ob], Act.Identity,
                                     bias=c5[:, ob:ob + 1], scale=a5[:, ob:ob + 1])
                o5 = pf.tile([128, N], dt.float32, tag="o5")
                nc.vector.scalar_tensor_tensor(out=o5, in0=z5, scalar=0.2, in1=z5,
                                               op0=Alu.mult, op1=Alu.max)
                nc.sync.dma_start(out_d[ob * 128:(ob + 1) * 128, :], o5)

    nc.compile()
    return nc


def _get_compiled():
    if "nc" not in _CACHE:
        _CACHE["nc"] = _build()
    return _CACHE["nc"]


def _make_in_maps(inputs):
    x = np.ascontiguousarray(np.asarray(inputs["x"], dtype=np.float32))
    shared = {}
    Cs = [3, 64, 64, 128]
    for li in range(4):
        w = np.asarray(inputs[f"w{li+1}"], dtype=np.float32)
        C = Cs[li]
        shared[f"waT{li}"] = np.ascontiguousarray(w[:, :C].T)
        shared[f"wbmaT{li}"] = np.ascontiguousarray((w[:, C:] - w[:, :C]).T)
        shared[f"g{li}"] = np.ascontiguousarray(
            np.asarray(inputs[f"g{li+1}"], np.float32).reshape(-1, 1))
        shared[f"b{li}"] = np.ascontiguousarray(
            np.asarray(inputs[f"b{li+1}"], np.float32).reshape(-1, 1))
    shared["w5T"] = np.ascontiguousarray(np.asarray(inputs["w5"], np.float32).T)
    shared["g5"] = np.ascontiguousarray(np.asarray(inputs["g5"], np.float32).reshape(-1, 1))
    shared["b5"] = np.ascontiguousarray(np.asarray(inputs["b5"], np.float32).reshape(-1, 1))

    return [dict(shared, x0=np.ascontiguousarray(x[i])) for i in range(B)]


def kernel(**inputs):
    from concourse.bass_utils import run_bass_kernel_spmd

    nc = _get_compiled()
    in_maps = _make_in_maps(inputs)
    res = run_bass_kernel_spmd(nc, in_maps, core_ids=list(range(8)))
    out = np.stack([res.results[i]["out"] for i in range(B)]).astype(np.float32)
    return out



# revision 12
# speedup vs baseline: 1.5878x; 1.2842x over previous
"""DGCNN forward kernel for Trainium2 (8 NeuronCores, batch-parallel).

Strategy (per core = one sample of the batch):
  - kNN scores S[n,c] = 2<x_n,x_c> - ||x_c||^2 computed directly on PE via an
    augmented matmul  S = [x;1]^T @ [2x;-xx]  (row and column orientations are
    bit-identical, so thresholds are consistent).
  - top-20 per row via 3 rounds of DVE max8/max_index + match_replace.
  - EdgeConv decomposition: y[o,n,j] = u[o, idx[n,j]] + v[o,n] with
    u = wa@x (neighbor part), v = (wb-wa)@x (center part). max_j then needs a
    gather-max over u only; gathers run on GPSIMD (ap_gather), segmented max on
    DVE pools.
  - BN statistics (training-mode, over the whole batch) from the SAME gathered
    u' tiles: s[o,n] = sum_j u'[idx[n,j]] via DVE add-reduce, sum of u'^2 via
    ACT Square+accum; u' mean-centered to avoid fp32 catastrophic cancellation;
    global sums via a tiny AllReduce (syncBN) per layer.
  - Final 1x1 conv + BN + leaky relu, stats again via AllReduce.
"""

import numpy as np

B, C0, N = 8, 3, 2048
K = 20
EPS = 1e-5
LAYERS = [(3, 64), (64, 64), (64, 128), (128, 256)]  # (C_in, O)
NT = N // 128          # 16 row tiles
NCH = N // 512         # 4 matmul free-dim chunks
NEG = -1.0e38

_CACHE = {}


def _build():
    import concourse.bass as bass
    import concourse.mybir as mybir
    from concourse import bacc
    from concourse.tile import TileContext

    dt = mybir.dt
    Alu = mybir.AluOpType
    Act = mybir.ActivationFunctionType

    nc = bacc.Bacc("TRN2", target_bir_lowering=False, debug=False,
                   enable_asserts=False, num_devices=8)

    # ---------------- DRAM I/O ----------------
    x_in = nc.dram_tensor("x0", [C0, N], dt.float32, kind="ExternalInput").ap()
    waT, wbmaT, gv, bv = {}, {}, {}, {}
    for li, (C, O) in enumerate(LAYERS):
        waT[li] = nc.dram_tensor(f"waT{li}", [C, O], dt.float32, kind="ExternalInput").ap()
        wbmaT[li] = nc.dram_tensor(f"wbmaT{li}", [C, O], dt.float32, kind="ExternalInput").ap()
        gv[li] = nc.dram_tensor(f"g{li}", [O, 1], dt.float32, kind="ExternalInput").ap()
        bv[li] = nc.dram_tensor(f"b{li}", [O, 1], dt.float32, kind="ExternalInput").ap()
    w5T_d = nc.dram_tensor("w5T", [512, 1024], dt.float32, kind="ExternalInput").ap()
    g5_d = nc.dram_tensor("g5", [1024, 1], dt.float32, kind="ExternalInput").ap()
    b5_d = nc.dram_tensor("b5", [1024, 1], dt.float32, kind="ExternalInput").ap()
    out_d = nc.dram_tensor("out", [1024, N], dt.float32, kind="ExternalOutput").ap()

    def sb(name, shape, dtype=dt.float32):
        return nc.alloc_sbuf_tensor(name, list(shape), dtype).ap()

    with TileContext(nc) as tc:
        # ---------------- persistent SBUF ----------------
        h = [sb("h0", [128, N]), sb("h1", [128, N]),
             sb("h2", [128, N]), sb("h3", [128, N])]
        ones_row = sb("ones_row", [1, N])
        nc.vector.memset(ones_row, 1.0)
        # identity for PE transpose: ident[p, f] = (f - p == 0)
        ident = sb("ident", [128, 128])
        iota_fp = sb("iota_fp", [128, 128], dt.int32)
        nc.gpsimd.iota(iota_fp, pattern=[[1, 128]], base=0, channel_multiplier=-1)
        nc.vector.tensor_scalar(out=ident, in0=iota_fp, scalar1=0, scalar2=None,
                                op0=Alu.is_equal)

        # x of current layer lives in h-slices; layer0 input loaded separately
        x0_sb = sb("x0_sb", [C0, N])
        nc.sync.dma_start(x0_sb, x_in)

        w5T_sb = sb("w5T_sb", [128, 4 * 1024])   # 4 c-blocks side by side
        for cb in range(4):
            nc.sync.dma_start(w5T_sb[:, cb * 1024:(cb + 1) * 1024],
                              w5T_d[cb * 128:(cb + 1) * 128, :])

        # x2 needs its own base-partition-0 tensor (matmul operands must share base)
        x2_sb = sb("x2_sb", [64, N])
        x_of = {0: x0_sb[:, :], 1: h[0][0:64, :], 2: x2_sb[:, :], 3: h[1][:, :]}
        hdst = {0: h[0][0:64, :], 1: x2_sb[:, :], 2: h[1][:, :],
                3: None}  # layer3 output 256ch -> h2,h3

        for li, (C, O) in enumerate(LAYERS):
            xc = x_of[li]
            OT = (O + 127) // 128            # o-tiles
            ow = [min(128, O - ot * 128) for ot in range(OT)]

            with tc.tile_pool(name=f"pp{li}", bufs=1) as pp:
              with tc.tile_pool(name=f"pt{li}", bufs=3, space="PSUM") as psT:
                # ---- augmented operands ----
                b2x = pp.tile([C, N], dt.float32, tag="b2x")
                nc.scalar.mul(b2x, xc, 2.0)
                xsq = pp.tile([C, N], dt.float32, tag="zt")
                nc.vector.tensor_tensor(out=xsq, in0=xc, in1=xc, op=Alu.mult)
                ones_col = pp.tile([C, 1], dt.float32, tag="ones_col")
                nc.vector.memset(ones_col, 1.0)
                bnxx = pp.tile([1, N], dt.float32, tag="bnxx")
                for nch in range(NCH):
                    xxp = psT.tile([1, 512], dt.float32, tag="pt512")
                    nc.tensor.matmul(xxp, lhsT=ones_col, rhs=xsq[:, nch * 512:(nch + 1) * 512],
                                     start=True, stop=True)
                    nc.scalar.mul(bnxx[:, nch * 512:(nch + 1) * 512], xxp, -1.0)

                # ---- u, v, centering ----
                waT_sb = pp.tile([C, O], dt.float32, tag="waT_sb")
                nc.sync.dma_start(waT_sb, waT[li])
                wbmaT_sb = pp.tile([C, O], dt.float32, tag="wbmaT_sb")
                nc.sync.dma_start(wbmaT_sb, wbmaT[li])
                up = pp.tile([128, OT * N], dt.float32, tag="up")       # u' (centered), o-tiles side by side
                vv = pp.tile([128, OT * N], dt.float32, tag="vv")
                sm = pp.tile([128, 96], dt.float32, tag="sm")
                neg_mu = sm[:, 0:2]
                sum_v = sm[:, 2:4]
                sum_u = sm[:, 4:6]
                sum_q = sm[:, 6:8]
                sum_s = sm[:, 8:10]
                svs = sm[:, 10:12]
                sv2 = sm[:, 12:14]
                vbar = sm[:, 14:16]
                beta = sm[:, 16:18]
                t1a = sm[:, 18:20]
                S1p = sm[:, 20:22]
                tA = sm[:, 22:24]
                tBt = sm[:, 24:26]
                S2p = sm[:, 26:28]
                tC = sm[:, 28:30]
                tD = sm[:, 30:32]
                mean = sm[:, 32:34]
                e2 = sm[:, 34:36]
                varp = sm[:, 36:38]
                rec = sm[:, 38:40]
                rsq = sm[:, 40:42]
                aco = sm[:, 42:44]
                bi = sm[:, 44:46]
                gsb = sm[:, 46:48]
                bsb = sm[:, 48:50]
                ar_in = sm[:, 50:54]
                ar_out = sm[:, 54:58]
                for ot in range(OT):
                    w_ = ow[ot]
                    for nch in range(NCH):
                        upp = psT.tile([128, 512], dt.float32, tag="pt512")
                        nc.tensor.matmul(upp[0:w_, :], lhsT=waT_sb[:, ot * 128:ot * 128 + w_],
                                         rhs=xc[:, nch * 512:(nch + 1) * 512], start=True, stop=True)
                        nc.scalar.activation(up[0:w_, ot * N + nch * 512: ot * N + (nch + 1) * 512],
                                             upp[0:w_, :], Act.Copy)
                        vpp = psT.tile([128, 512], dt.float32, tag="pt512")
                        nc.tensor.matmul(vpp[0:w_, :], lhsT=wbmaT_sb[:, ot * 128:ot * 128 + w_],
                                         rhs=xc[:, nch * 512:(nch + 1) * 512], start=True, stop=True)
                        nc.scalar.activation(vv[0:w_, ot * N + nch * 512: ot * N + (nch + 1) * 512],
                                             vpp[0:w_, :], Act.Copy)
                    nc.vector.tensor_reduce(out=sum_u[0:w_, ot:ot + 1],
                                            in_=up[0:w_, ot * N:(ot + 1) * N],
                                            op=Alu.add, axis=mybir.AxisListType.X)
                    nc.vector.tensor_reduce(out=sum_v[0:w_, ot:ot + 1],
                                            in_=vv[0:w_, ot * N:(ot + 1) * N],
                                            op=Alu.add, axis=mybir.AxisListType.X)
                    nc.scalar.mul(neg_mu[0:w_, ot:ot + 1], sum_u[0:w_, ot:ot + 1], -1.0 / N)
                    # center u in place
                    nc.scalar.activation(up[0:w_, ot * N:(ot + 1) * N],
                                         up[0:w_, ot * N:(ot + 1) * N], Act.Identity,
                                         bias=neg_mu[0:w_, ot:ot + 1], scale=1.0)

                # ---- fused pipeline: scores+topk(rt) | lists(rt-2) | gather(rt-3) ----
                CH = 128 if O >= 128 else 64
                NG = CH // 16
                lg1 = pp.tile([CH, NT * 128], dt.int16, tag="lg1")
                lg2 = pp.tile([CH, NT * 64], dt.int16, tag="lg2")
                mm = pp.tile([128, OT * N], dt.float32, tag="mm")      # m' per o-tile
                s_sb = pp.tile([128, OT * N], dt.float32, tag="s_sb")  # s[o,n] = sum_j u'[idx]
                qac = pp.tile([128, OT * 64], dt.float32, tag="qac")   # per-chunk sq-sum accums
                CKR = 64                        # rows per gather chunk

                def emit_scores_topk(rt, paS, paI):
                    Ssb = paS.tile([128, N], dt.float32, tag="Ssb")
                    for nch in range(NCH):
                        Spc = psT.tile([128, 512], dt.float32, tag="pt512")
                        nc.tensor.matmul(Spc,
                                         lhsT=xc[:, rt * 128:(rt + 1) * 128],
                                         rhs=b2x[:, nch * 512:(nch + 1) * 512],
                                         start=True, stop=False)
                        nc.tensor.matmul(Spc,
                                         lhsT=ones_row[:, rt * 128:(rt + 1) * 128],
                                         rhs=bnxx[:, nch * 512:(nch + 1) * 512],
                                         start=False, stop=True)
                        nc.scalar.activation(Ssb[:, nch * 512:(nch + 1) * 512], Spc, Act.Copy)
                    V = paI.tile([128, 24], dt.float32, tag="V")
                    I = paI.tile([128, 24], dt.uint16, tag="I")
                    Sw = paS.tile([128, N], dt.float32, tag="Ssb")   # share slots with Ssb
                    nc.vector.max(out=V[:, 0:8], in_=Ssb)
                    nc.vector.max_index(out=I[:, 0:8], in_max=V[:, 0:8], in_values=Ssb)
                    nc.vector.match_replace(out=Sw, in_to_replace=V[:, 0:8],
                                            in_values=Ssb, imm_value=NEG)
                    nc.vector.max(out=V[:, 8:16], in_=Sw)
                    nc.vector.max_index(out=I[:, 8:16], in_max=V[:, 8:16], in_values=Sw)
                    nc.vector.match_replace(out=Sw, in_to_replace=V[:, 8:16],
                                            in_values=Sw, imm_value=NEG)
                    nc.vector.max(out=V[:, 16:24], in_=Sw)
                    nc.vector.max_index(out=I[:, 16:24], in_max=V[:, 16:24], in_values=Sw)
                    If = paI.tile([128, 24], dt.float32, tag="If")
                    nc.vector.tensor_copy(If, I)
                    return If

                def emit_lists(rt, If, paI):
                    # transpose index lists into the [neighbor-slot, point] layout
                    # ap_gather wants, directly into group 0 of lg1/lg2, then
                    # DMA-replicate to the other channel groups.
                    tpa = psT2.tile([128, 128], dt.float32, tag="ptT")
                    nc.tensor.transpose(tpa[0:16, :], If[:, 0:16], ident)
                    nc.vector.tensor_copy(lg1[0:16, rt * 128:(rt + 1) * 128], tpa[0:16, :])
                    tpb = psT2.tile([128, 128], dt.float32, tag="ptT")
                    nc.tensor.transpose(tpb[0:8, :], If[:, 16:24], ident)
                    st8 = paI.tile([8, 128], dt.int16, tag="st8")
                    for bb in range(2):
                        nc.vector.tensor_copy(st8[:, bb * 64:(bb + 1) * 64],
                                              tpb[0:8, bb::2])
                        nc.sync.dma_start(
                            lg2[bb * 8:(bb + 1) * 8, rt * 64:(rt + 1) * 64],
                            st8[:, bb * 64:(bb + 1) * 64])
                    for g in range(1, NG):
                        nc.sync.dma_start(lg1[g * 16:(g + 1) * 16, rt * 128:(rt + 1) * 128],
                                          lg1[0:16, rt * 128:(rt + 1) * 128])
                        nc.sync.dma_start(lg2[g * 16:(g + 1) * 16, rt * 64:(rt + 1) * 64],
                                          lg2[0:16, rt * 64:(rt + 1) * 64])

                def emit_gather(rt, pg):
                    for ot in range(OT):
                        w_ = ow[ot]
                        wch = ((w_ + 15) // 16) * 16
                        usrc = up[0:wch, ot * N:(ot + 1) * N]
                        for ck in (2 * rt, 2 * rt + 1):
                            g1 = pg.tile([CH, CKR * 16], dt.float32, tag="g1")
                            nc.gpsimd.ap_gather(
                                g1[0:wch, :], usrc,
                                lg1[0:wch, ck * CKR: (ck + 1) * CKR],
                                channels=wch, num_elems=N, d=1, num_idxs=CKR * 16)
                            nc.vector.tensor_reduce(
                                out=mm[0:w_, ot * N + ck * CKR: ot * N + (ck + 1) * CKR],
                                in_=g1[0:w_, :].rearrange("p (n k) -> p n k", k=16),
                                op=Alu.max, axis=mybir.AxisListType.X)
                            g2 = pg.tile([CH, CKR * 8], dt.float32, tag="g2")
                            nc.gpsimd.ap_gather(
                                g2[0:wch, :], usrc,
                                lg2[0:wch, ck * (CKR // 2): (ck + 1) * (CKR // 2)],
                                channels=wch, num_elems=N, d=1, num_idxs=CKR * 8)
                            m2 = pg.tile([128, CKR], dt.float32, tag="m2")
                            nc.vector.tensor_reduce(
                                out=m2[0:w_, :],
                                in_=g2[0:w_, :].rearrange("p (n k) -> p n k", k=8)[:, :, 0:4],
                                op=Alu.max, axis=mybir.AxisListType.X)
                            nc.vector.tensor_tensor(
                                out=mm[0:w_, ot * N + ck * CKR: ot * N + (ck + 1) * CKR],
                                in0=mm[0:w_, ot * N + ck * CKR: ot * N + (ck + 1) * CKR],
                                in1=m2[0:w_, :], op=Alu.max)
                            # s: sum over the 20 neighbors (16 from g1 + first 4 of g2)
                            s1 = pg.tile([128, CKR], dt.float32, tag="s1")
                            nc.vector.tensor_reduce(
                                out=s1[0:w_, :],
                                in_=g1[0:w_, :].rearrange("p (n k) -> p n k", k=16),
                                op=Alu.add, axis=mybir.AxisListType.X)
                            s2 = pg.tile([128, CKR], dt.float32, tag="s2")
                            nc.vector.tensor_reduce(
                                out=s2[0:w_, :],
                                in_=g2[0:w_, :].rearrange("p (n k) -> p n k", k=8)[:, :, 0:4],
                                op=Alu.add, axis=mybir.AxisListType.X)
                            nc.vector.tensor_tensor(
                                out=s_sb[0:w_, ot * N + ck * CKR: ot * N + (ck + 1) * CKR],
                                in0=s1[0:w_, :], in1=s2[0:w_, :], op=Alu.add)
                            # q: global sum of u'^2 over same neighbors (ACT square+
                            # accum, in place — g1/g2 have no readers after the
                            # reduces above, so the WAR dep just orders the ops)
                            nc.scalar.activation(
                                g1[0:w_, :], g1[0:w_, :], Act.Square,
                                accum_out=qac[0:w_, ot * 64 + 2 * ck: ot * 64 + 2 * ck + 1])
                            g2v = g2[0:w_, :].rearrange("p (n k) -> p n k", k=8)[:, :, 0:4]
                            nc.scalar.activation(
                                g2v, g2v, Act.Square,
                                accum_out=qac[0:w_, ot * 64 + 2 * ck + 1: ot * 64 + 2 * ck + 2])

                with tc.tile_pool(name=f"paS{li}", bufs=2) as paS, \
                     tc.tile_pool(name=f"paI{li}", bufs=4) as paI, \
                     tc.tile_pool(name=f"pg{li}", bufs=2) as pg, \
                     tc.tile_pool(name=f"ptT{li}", bufs=4, space="PSUM") as psT2:
                    Ifs = {}
                    for it in range(NT + 3):
                        if it < NT:
                            Ifs[it] = emit_scores_topk(it, paS, paI)
                        if 0 <= it - 2 < NT:
                            emit_lists(it - 2, Ifs.pop(it - 2), paI)
                        if 0 <= it - 3 < NT:
                            emit_gather(it - 3, pg)
                for ot in range(OT):
                    w_ = ow[ot]
                    nc.vector.tensor_reduce(out=sum_q[0:w_, ot:ot + 1],
                                            in_=qac[0:w_, ot * 64:(ot + 1) * 64],
                                            op=Alu.add, axis=mybir.AxisListType.X)

                # ---- per-core stat terms + AllReduce ----
                for ot in range(OT):
                    w_ = ow[ot]
                    ssl = s_sb[0:w_, ot * N:(ot + 1) * N]
                    vsl = vv[0:w_, ot * N:(ot + 1) * N]
                    nc.vector.tensor_reduce(out=sum_s[0:w_, ot:ot + 1], in_=ssl,
                                            op=Alu.add, axis=mybir.AxisListType.X)
                    junk = pp.tile([128, N], dt.float32, tag="zt")
                    nc.vector.scalar_tensor_tensor(out=junk[0:w_, :], in0=ssl, scalar=1.0,
                                                   in1=vsl, op0=Alu.mult, op1=Alu.mult,
                                                   accum_out=svs[0:w_, ot:ot + 1])
                    nc.vector.scalar_tensor_tensor(out=junk[0:w_, :], in0=vsl, scalar=1.0,
                                                   in1=vsl, op0=Alu.mult, op1=Alu.mult,
                                                   accum_out=sv2[0:w_, ot:ot + 1])
                    # small [w_,1] algebra on DVE/ACT:
                    nc.scalar.mul(vbar[0:w_, ot:ot + 1], sum_v[0:w_, ot:ot + 1], 1.0 / N)
                    nc.vector.scalar_tensor_tensor(out=beta[0:w_, ot:ot + 1],
                                                   in0=neg_mu[0:w_, ot:ot + 1], scalar=-1.0,
                                                   in1=vbar[0:w_, ot:ot + 1],
                                                   op0=Alu.mult, op1=Alu.add)
                    # S1' = sum_s + K*(sum_v - N*vbar);  sum_v - N*vbar == 0 exactly? keep it:
                    nc.vector.scalar_tensor_tensor(out=t1a[0:w_, ot:ot + 1],
                                                   in0=vbar[0:w_, ot:ot + 1], scalar=-float(N),
                                                   in1=sum_v[0:w_, ot:ot + 1],
                                                   op0=Alu.mult, op1=Alu.add)  # sum_v - N*vbar
                    nc.vector.scalar_tensor_tensor(out=S1p[0:w_, ot:ot + 1],
                                                   in0=t1a[0:w_, ot:ot + 1], scalar=float(K),
                                                   in1=sum_s[0:w_, ot:ot + 1],
                                                   op0=Alu.mult, op1=Alu.add)
                    # S2' = sum_q + 2*(svs - vbar*sum_s) + K*(sv2 - N*vbar^2)
                    nc.vector.tensor_tensor(out=tA[0:w_, ot:ot + 1], in0=vbar[0:w_, ot:ot + 1],
                                            in1=sum_s[0:w_, ot:ot + 1], op=Alu.mult)
                    nc.vector.scalar_tensor_tensor(out=tA[0:w_, ot:ot + 1],
                                                   in0=tA[0:w_, ot:ot + 1], scalar=-1.0,
                                                   in1=svs[0:w_, ot:ot + 1],
                                                   op0=Alu.mult, op1=Alu.add)  # svs - vbar*sum_s
                    nc.vector.tensor_tensor(out=tBt[0:w_, ot:ot + 1], in0=vbar[0:w_, ot:ot + 1],
                                            in1=vbar[0:w_, ot:ot + 1], op=Alu.mult)
                    nc.vector.scalar_tensor_tensor(out=tBt[0:w_, ot:ot + 1],
                                                   in0=tBt[0:w_, ot:ot + 1], scalar=-float(N),
                                                   in1=sv2[0:w_, ot:ot + 1],
                                                   op0=Alu.mult, op1=Alu.add)  # sv2 - N*vbar^2
                    nc.vector.scalar_tensor_tensor(out=S2p[0:w_, ot:ot + 1],
                                                   in0=tA[0:w_, ot:ot + 1], scalar=2.0,
                                                   in1=sum_q[0:w_, ot:ot + 1],
                                                   op0=Alu.mult, op1=Alu.add)
                    nc.vector.scalar_tensor_tensor(out=S2p[0:w_, ot:ot + 1],
                                                   in0=tBt[0:w_, ot:ot + 1], scalar=float(K),
                                                   in1=S2p[0:w_, ot:ot + 1],
                                                   op0=Alu.mult, op1=Alu.add)
                    # t1 = S1' + cnt*beta ; t2 = S2' + 2*beta*S1' + cnt*beta^2
                    cntl = float(N * K)
                    nc.vector.scalar_tensor_tensor(out=ar_in[0:w_, 2 * ot:2 * ot + 1],
                                                   in0=beta[0:w_, ot:ot + 1], scalar=cntl,
                                                   in1=S1p[0:w_, ot:ot + 1],
                                                   op0=Alu.mult, op1=Alu.add)
                    nc.vector.tensor_tensor(out=tC[0:w_, ot:ot + 1], in0=beta[0:w_, ot:ot + 1],
                                            in1=S1p[0:w_, ot:ot + 1], op=Alu.mult)
                    nc.vector.scalar_tensor_tensor(out=tC[0:w_, ot:ot + 1],
                                                   in0=tC[0:w_, ot:ot + 1], scalar=2.0,
                                                   in1=S2p[0:w_, ot:ot + 1],
                                                   op0=Alu.mult, op1=Alu.add)
                    nc.vector.tensor_tensor(out=tD[0:w_, ot:ot + 1], in0=beta[0:w_, ot:ot + 1],
                                            in1=beta[0:w_, ot:ot + 1], op=Alu.mult)
                    nc.vector.scalar_tensor_tensor(out=ar_in[0:w_, 2 * ot + 1:2 * ot + 2],
                                                   in0=tD[0:w_, ot:ot + 1], scalar=cntl,
                                                   in1=tC[0:w_, ot:ot + 1],
                                                   op0=Alu.mult, op1=Alu.add)

                with tc.tile_pool(name=f"dr{li}", bufs=1, space="DRAM") as dram:
                    ari = dram.tile([128, 2 * OT], dt.float32)
                    aro = dram.tile([128, 2 * OT], dt.float32)
                    nc.sync.dma_start(ari[:], ar_in[:, 0:2 * OT])
                    nc.gpsimd.collective_compute(
                        "AllReduce", Alu.add, replica_groups=[list(range(8))],
                        ins=[ari.opt()], outs=[aro.opt()])
                    nc.sync.dma_start(ar_out[:, 0:2 * OT], aro[:])

                # post-AR: mean/var/scale/bias + activation
                nc.sync.dma_start(gsb[0:ow[0], 0:1], gv[li][0:ow[0], :])
                nc.sync.dma_start(bsb[0:ow[0], 0:1], bv[li][0:ow[0], :])
                if OT > 1:
                    nc.sync.dma_start(gsb[0:ow[1], 1:2], gv[li][128:128 + ow[1], :])
                    nc.sync.dma_start(bsb[0:ow[1], 1:2], bv[li][128:128 + ow[1], :])
                cntg = float(B * N * K)
                for ot in range(OT):
                    w_ = ow[ot]
                    nc.scalar.mul(mean[0:w_, ot:ot + 1], ar_out[0:w_, 2 * ot:2 * ot + 1], 1.0 / cntg)
                    nc.scalar.mul(e2[0:w_, ot:ot + 1], ar_out[0:w_, 2 * ot + 1:2 * ot + 2], 1.0 / cntg)
                    nc.vector.tensor_tensor(out=varp[0:w_, ot:ot + 1], in0=mean[0:w_, ot:ot + 1],
                                            in1=mean[0:w_, ot:ot + 1], op=Alu.mult)
                    nc.vector.scalar_tensor_tensor(out=varp[0:w_, ot:ot + 1],
                                                   in0=varp[0:w_, ot:ot + 1], scalar=-1.0,
                                                   in1=e2[0:w_, ot:ot + 1],
                                                   op0=Alu.mult, op1=Alu.add)
                    nc.vector.tensor_scalar_add(varp[0:w_, ot:ot + 1], varp[0:w_, ot:ot + 1], EPS)
                    nc.vector.reciprocal(rec[0:w_, ot:ot + 1], varp[0:w_, ot:ot + 1])
                    nc.scalar.sqrt(rsq[0:w_, ot:ot + 1], rec[0:w_, ot:ot + 1])
                    nc.vector.tensor_tensor(out=aco[0:w_, ot:ot + 1], in0=gsb[0:w_, ot:ot + 1],
                                            in1=rsq[0:w_, ot:ot + 1], op=Alu.mult)
                    # bias2 = b + a*(mu_u - mean) = b - a*(neg_mu + mean)
                    nc.vector.tensor_tensor(out=bi[0:w_, ot:ot + 1], in0=neg_mu[0:w_, ot:ot + 1],
                                            in1=mean[0:w_, ot:ot + 1], op=Alu.add)
                    nc.vector.tensor_tensor(out=bi[0:w_, ot:ot + 1], in0=bi[0:w_, ot:ot + 1],
                                            in1=aco[0:w_, ot:ot + 1], op=Alu.mult)
                    nc.vector.scalar_tensor_tensor(out=bi[0:w_, ot:ot + 1],
                                                   in0=bi[0:w_, ot:ot + 1], scalar=-1.0,
                                                   in1=bsb[0:w_, ot:ot + 1],
                                                   op0=Alu.mult, op1=Alu.add)
                    # z = a*(m' + v) + bias2 ; lrelu
                    if li < 3:
                        dst = hdst[li]
                    else:
                        dst = h[2][:, :] if ot == 0 else h[3][:, :]
                    dsl = dst if li == 3 else dst
                    zt = pp.tile([128, N], dt.float32, tag="zt")
                    nc.vector.tensor_tensor(out=zt[0:w_, :], in0=mm[0:w_, ot * N:(ot + 1) * N],
                                            in1=vv[0:w_, ot * N:(ot + 1) * N], op=Alu.add)
                    nc.scalar.activation(zt[0:w_, :], zt[0:w_, :], Act.Identity,
                                         bias=bi[0:w_, ot:ot + 1], scale=aco[0:w_, ot:ot + 1])
                    nc.vector.scalar_tensor_tensor(out=dsl[0:w_, :] if li == 3 else dst[0:w_, :],
                                                   in0=zt[0:w_, :], scalar=0.2,
                                                   in1=zt[0:w_, :], op0=Alu.mult, op1=Alu.max)
                    if li == 1:
                        nc.sync.dma_start(h[0][64:128, :], x2_sb)

        # ---------------- final conv + BN + lrelu ----------------
        with tc.tile_pool(name="pf", bufs=1) as pf, \
             tc.tile_pool(name="pfp", bufs=2, space="PSUM") as pfp:
            y_sb = []
            for ob in range(8):
                ytile = pf.tile([128, N], dt.float32, tag=f"y{ob}")
                y_sb.append(ytile)
            sm5 = pf.tile([128, 160], dt.float32, tag="sm5")
            sum_y = sm5[:, 0:8]
            mu5 = sm5[:, 8:16]
            nmu5 = sm5[:, 16:24]
            syc2 = sm5[:, 24:32]
            tE = sm5[:, 32:40]
            tF = sm5[:, 40:48]
            g5_sb = sm5[:, 48:56]
            b5_sb = sm5[:, 56:64]
            mean5 = sm5[:, 64:72]
            e25 = sm5[:, 72:80]
            var5 = sm5[:, 80:88]
            rec5 = sm5[:, 88:96]
            rsq5 = sm5[:, 96:104]
            a5 = sm5[:, 104:112]
            c5 = sm5[:, 112:120]
            ar5_in = sm5[:, 120:136]
            ar5_out = sm5[:, 136:152]
            for ob in range(8):
                for nch in range(NCH):
                    yp = pfp.tile([128, 512], dt.float32, tag="yp")
                    for cb in range(4):
                        nc.tensor.matmul(yp, lhsT=w5T_sb[:, cb * 1024 + ob * 128:
                                                         cb * 1024 + (ob + 1) * 128],
                                         rhs=h[cb][:, nch * 512:(nch + 1) * 512],
                                         start=(cb == 0), stop=(cb == 3))
                    nc.scalar.activation(y_sb[ob][:, nch * 512:(nch + 1) * 512], yp, Act.Copy)
                nc.vector.tensor_reduce(out=sum_y[:, ob:ob + 1], in_=y_sb[ob],
                                        op=Alu.add, axis=mybir.AxisListType.X)
                nc.scalar.mul(mu5[:, ob:ob + 1], sum_y[:, ob:ob + 1], 1.0 / N)
                nc.scalar.mul(nmu5[:, ob:ob + 1], sum_y[:, ob:ob + 1], -1.0 / N)
                yc = pf.tile([128, N], dt.float32, tag="yc")
                nc.scalar.activation(yc, y_sb[ob], Act.Identity,
                                     bias=nmu5[:, ob:ob + 1], scale=1.0)
                junk5 = pf.tile([128, N], dt.float32, tag="junk5")
                nc.vector.scalar_tensor_tensor(out=junk5, in0=yc, scalar=1.0, in1=yc,
                                               op0=Alu.mult, op1=Alu.mult,
                                               accum_out=syc2[:, ob:ob + 1])
                # t1 = sum_y ; t2 = syc2 + 2*mu5*(sum_y - N*mu5) + N*mu5^2
                #    = syc2 + 2*mu5*sum_y - N*mu5^2
                nc.vector.tensor_copy(ar5_in[:, 2 * ob:2 * ob + 1], sum_y[:, ob:ob + 1])
                nc.vector.tensor_tensor(out=tE[:, ob:ob + 1], in0=mu5[:, ob:ob + 1],
                                        in1=sum_y[:, ob:ob + 1], op=Alu.mult)
                nc.vector.scalar_tensor_tensor(out=tE[:, ob:ob + 1], in0=tE[:, ob:ob + 1],
                                               scalar=2.0, in1=syc2[:, ob:ob + 1],
                                               op0=Alu.mult, op1=Alu.add)
                nc.vector.tensor_tensor(out=tF[:, ob:ob + 1], in0=mu5[:, ob:ob + 1],
                                        in1=mu5[:, ob:ob + 1], op=Alu.mult)
                nc.vector.scalar_tensor_tensor(out=ar5_in[:, 2 * ob + 1:2 * ob + 2],
                                               in0=tF[:, ob:ob + 1], scalar=-float(N),
                                               in1=tE[:, ob:ob + 1],
                                               op0=Alu.mult, op1=Alu.add)
            with tc.tile_pool(name="dr5", bufs=1, space="DRAM") as dram5:
                ari5 = dram5.tile([128, 16], dt.float32)
                aro5 = dram5.tile([128, 16], dt.float32)
                nc.sync.dma_start(ari5[:], ar5_in)
                nc.gpsimd.collective_compute(
                    "AllReduce", Alu.add, replica_groups=[list(range(8))],
                    ins=[ari5.opt()], outs=[aro5.opt()])
                nc.sync.dma_start(ar5_out, aro5[:])
            for ob in range(8):
                nc.sync.dma_start(g5_sb[:, ob:ob + 1], g5_d[ob * 128:(ob + 1) * 128, :])
                nc.sync.dma_start(b5_sb[:, ob:ob + 1], b5_d[ob * 128:(ob + 1) * 128, :])
            cnt5 = float(B * N)
            for ob in range(8):
                nc.scalar.mul(mean5[:, ob:ob + 1], ar5_out[:, 2 * ob:2 * ob + 1], 1.0 / cnt5)
                nc.scalar.mul(e25[:, ob:ob + 1], ar5_out[:, 2 * ob + 1:2 * ob + 2], 1.0 / cnt5)
                nc.vector.tensor_tensor(out=var5[:, ob:ob + 1], in0=mean5[:, ob:ob + 1],
                                        in1=mean5[:, ob:ob + 1], op=Alu.mult)
                nc.vector.scalar_tensor_tensor(out=var5[:, ob:ob + 1], in0=var5[:, ob:ob + 1],
                                               scalar=-1.0, in1=e25[:, ob:ob + 1],
                                               op0=Alu.mult, op1=Alu.add)
                nc.vector.tensor_scalar_add(var5[:, ob:ob + 1], var5[:, ob:ob + 1], EPS)
                nc.vector.reciprocal(rec5[:, ob:ob + 1], var5[:, ob:ob + 1])
                nc.scalar.sqrt(rsq5[:, ob:ob + 1], rec5[:, ob:ob + 1])
                nc.vector.tensor_tensor(out=a5[:, ob:ob + 1], in0=g5_sb[:, ob:ob + 1],
                                        in1=rsq5[:, ob:ob + 1], op=Alu.mult)
                nc.vector.tensor_tensor(out=c5[:, ob:ob + 1], in0=mean5[:, ob:ob + 1],
                                        in1=a5[:, ob:ob + 1], op=Alu.mult)
                nc.vector.scalar_tensor_tensor(out=c5[:, ob:ob + 1], in0=c5[:, ob:ob + 1],
                                               scalar=-1.0, in1=b5_sb[:, ob:ob + 1],
                                               op0=Alu.mult, op1=Alu.add)
                z5 = pf.tile([128, N], dt.float32, tag="z5")
                nc.scalar.activation(z5, y_sb[ob], Act.Identity,
                                     bias=c5[:, ob:ob + 1], scale=a5[:, ob:ob + 1])
                o5 = pf.tile([128, N], dt.float32, tag="o5")
                nc.vector.scalar_tensor_tensor(out=o5, in0=z5, scalar=0.2, in1=z5,
                                               op0=Alu.mult, op1=Alu.max)
                nc.sync.dma_start(out_d[ob * 128:(ob + 1) * 128, :], o5)

    nc.compile()
    return nc


def _get_compiled():
    if "nc" not in _CACHE:
        _CACHE["nc"] = _build()
    return _CACHE["nc"]


def _make_in_maps(inputs):
    x = np.ascontiguousarray(np.asarray(inputs["x"], dtype=np.float32))
    shared = {}
    Cs = [3, 64, 64, 128]
    for li in range(4):
        w = np.asarray(inputs[f"w{li+1}"], dtype=np.float32)
        C = Cs[li]
        shared[f"waT{li}"] = np.ascontiguousarray(w[:, :C].T)
        shared[f"wbmaT{li}"] = np.ascontiguousarray((w[:, C:] - w[:, :C]).T)
        shared[f"g{li}"] = np.ascontiguousarray(
            np.asarray(inputs[f"g{li+1}"], np.float32).reshape(-1, 1))
        shared[f"b{li}"] = np.ascontiguousarray(
            np.asarray(inputs[f"b{li+1}"], np.float32).reshape(-1, 1))
    shared["w5T"] = np.ascontiguousarray(np.asarray(inputs["w5"], np.float32).T)
    shared["g5"] = np.ascontiguousarray(np.asarray(inputs["g5"], np.float32).reshape(-1, 1))
    shared["b5"] = np.ascontiguousarray(np.asarray(inputs["b5"], np.float32).reshape(-1, 1))

    return [dict(shared, x0=np.ascontiguousarray(x[i])) for i in range(B)]


def kernel(**inputs):
    from concourse.bass_utils import run_bass_kernel_spmd

    nc = _get_compiled()
    in_maps = _make_in_maps(inputs)
    res = run_bass_kernel_spmd(nc, in_maps, core_ids=list(range(8)))
    out = np.stack([res.results[i]["out"] for i in range(B)]).astype(np.float32)
    return out

